# revision 27
# baseline (speedup 1.0000x reference)
"""Trainium2 Bass kernel for slot-routed classifier head (moe_routing).

Reference computation (per token t, slot s = t % 30):
    logits = x[t] @ W[s] + b[s]            # [200]
    nll[t] = logsumexp(logits) - logits[id[t]]
    loss   = mean(nll over valid tokens);  preds[t] = argmax(logits)

Strategy:
  - Host: reorder X to slot-major transposed layout [30, 768, B_CORE] per core
    (data-parallel over the 4096 dialogs, 512 dialogs per core). This makes
    every matmul operand naturally laid out (contraction dim on partitions),
    so the device does zero transposes.
  - Device (per core): for each (slot, dialog-tile of 128):
      PE:  6 accumulating fp32 matmuls -> PSUM logits [128 tok, 200 cls]
      DVE: bias-add + PSUM evac (scalar_tensor_tensor)
           reduce_max -> m
           label gather: (iota == id) * logits, fused free-dim accum -> logit[id]
           argmax: (logits >= m) * (200 - iota), reduce_max -> 200 - argmax
      ACT: Exp with fused free-dim accum -> Z (no max-sub needed: |logits| < ~6)
    Tails: Ln(Z), nll = lnZ - label_logit, preds = 200 - mx; DMA out.
  - Host: mask invalid (-1) ids, fp64 sum for the scalar loss, gather preds.
"""

import os
from contextlib import ExitStack

import numpy as np

import concourse.bass as bass
import concourse.mybir as mybir
import concourse.tile as tile
from concourse.bass_utils import run_bass_kernel_spmd

# Problem constants (hardcoded per contract)
N_SLOTS = 30
HIDDEN = 768
C = 200  # num labels
N_TOKENS = 122880
N_CORES = 8
B = N_TOKENS // N_SLOTS  # 4096 dialogs
P = 128
KC = HIDDEN // P  # 6 contraction chunks

F32 = mybir.dt.float32
F16 = mybir.dt.float16
I32 = mybir.dt.int32
AX = mybir.AxisListType
OP = mybir.AluOpType
AF = mybir.ActivationFunctionType

# fp16 hi/lo split matmul: logits are computed at scale WSCALE (W pre-scaled
# on host to dodge fp16 subnormals); exp/label paths rescale by 1/WSCALE.
WSCALE = 64.0


def _build(b_core: int) -> bass.Bass:
    """Build the single-core SPMD program for a shard of b_core dialogs."""
    dt_n = b_core // P  # dialog tiles per core
    ncols = dt_n * N_SLOTS

    nc = bass.Bass("TRN2", target_bir_lowering=False, debug=False)

    # xw packs [Xhi | Xlo | W'hi | W'lo] (fp16) along the free dim:
    # one DMA (and one wait) per slot
    fw = 2 * b_core + 2 * C
    xw = nc.dram_tensor("xw", [N_SLOTS, HIDDEN, fw], F16, kind="ExternalInput").ap()
    bb = nc.dram_tensor("bb", [P, N_SLOTS, C], F32, kind="ExternalInput").ap()
    ids = nc.dram_tensor("ids", [P, ncols], F32, kind="ExternalInput").ap()
    io = nc.dram_tensor("io", [P, 2, C], F32, kind="ExternalInput").ap()
    nll = nc.dram_tensor("nll", [P, ncols], F32, kind="ExternalOutput").ap()
    preds = nc.dram_tensor("preds", [P, ncols], I32, kind="ExternalOutput").ap()

    with tile.TileContext(nc) as tc, ExitStack() as ctx:
        const_pool = ctx.enter_context(tc.tile_pool(name="const", bufs=1))
        xt_pool = ctx.enter_context(tc.tile_pool(name="xtp", bufs=2))
        psum_pool = ctx.enter_context(tc.tile_pool(name="ps", bufs=6, space="PSUM"))
        lg_pool = ctx.enter_context(tc.tile_pool(name="lg", bufs=4))
        e_pool = ctx.enter_context(tc.tile_pool(name="ep", bufs=2))
        j_pool = ctx.enter_context(tc.tile_pool(name="jp", bufs=4))

        # Constants / accumulators (persistent)
        bb_sb = const_pool.tile([P, N_SLOTS, C], F32)
        nc.sync.dma_start(bb_sb[:], bb[:])
        ids_sb = const_pool.tile([P, ncols], F32)
        nc.sync.dma_start(ids_sb[:], ids[:])
        io_sb = const_pool.tile([P, 2, C], F32)
        nc.sync.dma_start(io_sb[:], io[:])

        # One-time wait absorbers: the S2S2D2_STT struct encodes only one sem
        # wait, so pre-touch each const DMA on DVE; later DVE ops then only
        # ever wait on the PE semaphore.
        warm = const_pool.tile([P, 4], F32)
        nc.vector.tensor_copy(warm[:, 0:1], bb_sb[:, 0, 0:1])
        nc.vector.tensor_copy(warm[:, 1:2], ids_sb[:, 0:1])
        nc.vector.tensor_copy(warm[:, 2:3], io_sb[:, 0, 0:1])

        m_all = const_pool.tile([P, ncols], F32)
        z_all = const_pool.tile([P, ncols], F32)
        lbl_all = const_pool.tile([P, ncols], F32)
        mx_all = const_pool.tile([P, ncols], F32)

        o_xlo = b_core
        o_whi = 2 * b_core
        o_wlo = 2 * b_core + C
        for s in range(N_SLOTS):
            xw_sb = xt_pool.tile([P, KC, fw], F16)
            nc.sync.dma_start(xw_sb[:], xw[s].rearrange("(k p) f -> p k f", p=P))

            for dti in range(dt_n):
                col = dti * N_SLOTS + s
                ps = psum_pool.tile([P, C], F32)
                for k in range(KC):
                    xhi = xw_sb[:, k, bass.ts(dti, P)]
                    xlo = xw_sb[:, k, o_xlo + dti * P : o_xlo + (dti + 1) * P]
                    whi = xw_sb[:, k, o_whi : o_whi + C]
                    wlo = xw_sb[:, k, o_wlo : o_wlo + C]
                    # logits*WSCALE = Xhi@W'hi + Xhi@W'lo + Xlo@W'hi
                    nc.tensor.matmul(ps[:], xhi, whi, start=(k == 0), stop=False)
                    nc.tensor.matmul(ps[:], xhi, wlo, start=False, stop=False)
                    nc.tensor.matmul(
                        ps[:], xlo, whi, start=False, stop=(k == KC - 1)
                    )
                # logits = psum + b_s   (bias add + PSUM evacuation in one op)
                lg = lg_pool.tile([P, C], F32)
                nc.vector.scalar_tensor_tensor(
                    out=lg[:], in0=ps[:], scalar=0.0, op0=OP.bypass,
                    in1=bb_sb[:, s, :], op1=OP.add,
                )
                # row max (for argmax)
                nc.vector.reduce_max(m_all[:, col : col + 1], lg[:], axis=AX.X)
                # e = exp(logits); Z = sum(e) fused
                ej = e_pool.tile([P, C], F32)
                nc.scalar.activation(
                    ej[:], lg[:], AF.Exp, scale=1.0 / WSCALE,
                    accum_out=z_all[:, col : col + 1],
                )
                # label logit: sum((iota == id) * logits)
                j1 = j_pool.tile([P, C], F32)
                nc.vector.scalar_tensor_tensor(
                    out=j1[:], in0=io_sb[:, 0, :], scalar=ids_sb[:, col : col + 1],
                    op0=OP.is_equal, in1=lg[:], op1=OP.mult,
                    accum_out=lbl_all[:, col : col + 1],
                )
                # argmax: max((logits >= m) * (200 - iota)) = 200 - argmax
                # (mask computed on the otherwise-idle GPSIMD engine; Pool
                # rejects TensorScalarPtr, so split into TS + TT)
                j2a = j_pool.tile([P, C], F32)
                nc.gpsimd.tensor_scalar(
                    j2a[:], lg[:], m_all[:, col : col + 1], None, op0=OP.is_ge
                )
                j2 = j_pool.tile([P, C], F32)
                nc.gpsimd.tensor_tensor(j2[:], j2a[:], io_sb[:, 1, :], op=OP.mult)
                nc.vector.reduce_max(mx_all[:, col : col + 1], j2[:], axis=AX.X)

        # Tails  (lbl_all is at scale WSCALE; nll = lnZ - lbl/WSCALE)
        lnz = const_pool.tile([P, ncols], F32)
        nc.scalar.activation(lnz[:], z_all[:], AF.Ln)
        nll_sb = const_pool.tile([P, ncols], F32)
        nc.vector.scalar_tensor_tensor(
            out=nll_sb[:], in0=lbl_all[:], scalar=-1.0 / WSCALE, op0=OP.mult,
            in1=lnz[:], op1=OP.add,
        )
        preds_sb = const_pool.tile([P, ncols], I32)
        nc.vector.tensor_scalar(
            preds_sb[:], mx_all[:], -1.0, 200.0, op0=OP.mult, op1=OP.add
        )
        nc.sync.dma_start(nll[:], nll_sb[:])
        nc.sync.dma_start(preds[:], preds_sb[:])

    _legalize_waits(nc)
    _split_residual_waits_in_json(nc)
    return nc


# ISA structs for compute ops encode a single sem-wait command; Tile's
# scheduler freely attaches several. Legalize: (1) drop waits on an engine's
# own sem that program order already satisfies, (2) hoist extra waits onto an
# earlier same-engine instruction with a free wait slot (safe: engines are
# in-order, so waiting earlier only strengthens the schedule).
_ONE_WAIT_OPS = {
    "Matmult", "Activation", "TensorScalarPtr", "TensorReduce", "TensorTensor",
    "TensorCopy", "TensorScalar", "Memset", "Iota", "TensorMaskReduce",
    "DMACopy",
}


def _legalize_waits(nc: bass.Bass, one_wait_ops=_ONE_WAIT_OPS):
    import bass_rust
    for f in nc.m.functions:
        for bl in f.blocks:
            insns = list(bl.instructions)
            sem_updaters: dict[int, set] = {}
            sem_async: set[int] = set()  # sems inc'd at async DMA completion
            for ins in insns:
                si = ins.sync_info
                if si:
                    for u in si.on_update:
                        sem_updaters.setdefault(u.id, set()).add(ins.engine)
                        if ins.opcode == "DMACopy":
                            sem_async.add(u.id)

            # sems incremented exclusively by one engine's (sync) instructions
            sync_engine_sem: dict[int, object] = {}
            for sid, ups in sem_updaters.items():
                if len(ups) == 1 and sid not in sem_async:
                    sync_engine_sem[sid] = next(iter(ups))

            # Pass A: implication tables. For each sync-engine sem S, for each
            # inc index n, the max (other_sem -> wait_value) seen on S's
            # engine stream before the n-th inc completes.
            # impl[S] = list of (inc_index, {sem: maxval}) snapshots.
            running: dict[object, dict[int, int]] = {}
            impl: dict[int, list] = {}
            inc_count: dict[int, int] = {}
            for ins in insns:
                eng = ins.engine
                si = ins.sync_info
                if not si:
                    continue
                r = running.setdefault(eng, {})
                for w in si.on_wait:
                    if w.wait_mode == "sem-ge-imm":
                        if r.get(w.id, -1) < w.wait_value:
                            r[w.id] = w.wait_value
                for u in si.on_update:
                    if u.update_mode == "sem-inc" and sync_engine_sem.get(u.id) == eng:
                        inc_count[u.id] = inc_count.get(u.id, 0) + u.update_value
                        impl.setdefault(u.id, []).append(
                            (inc_count[u.id], dict(r))
                        )

            def implied(kept_waits, w):
                """Is wait w implied by any wait already kept?"""
                for k in kept_waits:
                    tab = impl.get(k.id)
                    if not tab or k.wait_mode != "sem-ge-imm":
                        continue
                    # snapshot at the largest inc index <= k.wait_value
                    snap = None
                    for n, d in tab:
                        if n <= k.wait_value:
                            snap = d
                        else:
                            break
                    if snap is not None and snap.get(w.id, -1) >= w.wait_value:
                        return True
                return False

            # Pass B: rewrite
            inc_seen: dict[int, int] = {}
            targets: dict = {}
            unplaced = 0
            for ins in insns:
                eng = ins.engine
                si = ins.sync_info
                waits = list(si.on_wait) if si else []
                updates = list(si.on_update) if si else []
                if ins.opcode in one_wait_ops and len(waits) > 1:
                    kept = []
                    for w in waits:
                        if (
                            sync_engine_sem.get(w.id) == eng
                            and w.wait_mode == "sem-ge-imm"
                            and inc_seen.get(w.id, 0) >= w.wait_value
                        ):
                            continue  # program order satisfies it
                        kept.append(w)
                    if len(kept) > 1:
                        primary = kept[:1]
                        for w in kept[1:]:
                            if not implied(primary, w):
                                primary.append(w)
                        kept = primary
                    # leftovers with >1 wait are split into preceding Drains
                    # at the JSON level (_split_residual_waits_in_json)
                    ins.sync_info = bass_rust.SyncInfo(on_wait=kept, on_update=updates)
                for u in updates:
                    if u.update_mode == "sem-inc":
                        inc_seen[u.id] = inc_seen.get(u.id, 0) + u.update_value
                if ins.opcode not in ("UnconditionalBranch", "Call"):
                    cur = ins.sync_info
                    if ins.opcode == "Drain" or not cur or len(cur.on_wait) == 0:
                        targets.setdefault(eng, []).append(ins)
                        if len(targets[eng]) > 64:
                            targets[eng] = targets[eng][-64:]
            if unplaced:
                import logging
                logging.warning(f"legalize_waits: {unplaced} waits could not be placed")


def _split_residual_waits_in_json(nc: bass.Bass):
    """Final walrus-facing fix: split any instruction still carrying >1 sem
    wait into preceding single-wait Drains on the same engine (pure wait
    sequencing — no reordering), and pin the patched JSON onto the instance
    so every downstream serialization (bass2jax, compile) uses it."""
    import orjson

    bir = orjson.loads(type(nc).to_json_bytes(nc))
    n = 0
    for f in bir["functions"]:
        for bl in f["blocks"]:
            out = []
            for ins in bl.get("instructions", []):
                si = ins.get("sync_info") or {}
                waits = si.get("on_wait") or []
                if len(waits) > 1:
                    for w in waits[:-1]:
                        n += 1
                        out.append({
                            "debug": ins.get("debug", 0),
                            "engine": ins["engine"],
                            "ins": [],
                            "outs": [],
                            "name": f"{ins['name']}-lgw{n}",
                            "opcode": "Drain",
                            "sync_info": {"on_update": [], "on_wait": [w]},
                        })
                    si = dict(si)
                    si["on_wait"] = waits[-1:]
                    ins = dict(ins)
                    ins["sync_info"] = si
                out.append(ins)
            bl["instructions"] = out
    data = orjson.dumps(bir)
    nc.to_json_bytes = lambda: data  # type: ignore[method-assign]


_NC_CACHE: dict[int, bass.Bass] = {}


def _get_nc(b_core: int) -> bass.Bass:
    if b_core not in _NC_CACHE:
        _NC_CACHE[b_core] = _build(b_core)
    return _NC_CACHE[b_core]


def _prep_inputs(state_output, value_match_ids, W, b, n_cores: int):
    """Host-side sharding: slot-major transposed X, per-core in_maps."""
    b_core = B // n_cores
    dt_n = b_core // P
    x3 = np.ascontiguousarray(state_output, dtype=np.float32).reshape(B, N_SLOTS, HIDDEN)
    # [30, 768, 4096]: xt[s, d, dialog]
    xt_full = x3.transpose(1, 2, 0)
    xhi_full = xt_full.astype(np.float16)
    xlo_full = (xt_full - xhi_full.astype(np.float32)).astype(np.float16)

    ws = np.asarray(W, dtype=np.float32) * np.float32(WSCALE)
    whi = ws.astype(np.float16)
    wlo = (ws - whi.astype(np.float32)).astype(np.float16)

    bb_np = np.ascontiguousarray(
        np.broadcast_to(
            np.asarray(b, dtype=np.float32) * np.float32(WSCALE), (P, N_SLOTS, C)
        )
    )
    iota = np.arange(C, dtype=np.float32)
    io_np = np.ascontiguousarray(
        np.broadcast_to(np.stack([iota, C - iota]), (P, 2, C))
    )

    ids2d = np.asarray(value_match_ids).reshape(B, N_SLOTS)
    in_maps = []
    for c in range(n_cores):
        sl = slice(c * b_core, (c + 1) * b_core)
        xw_c = np.empty((N_SLOTS, HIDDEN, 2 * b_core + 2 * C), dtype=np.float16)
        xw_c[:, :, :b_core] = xhi_full[:, :, sl]
        xw_c[:, :, b_core : 2 * b_core] = xlo_full[:, :, sl]
        xw_c[:, :, 2 * b_core : 2 * b_core + C] = whi
        xw_c[:, :, 2 * b_core + C :] = wlo
        ids_c = ids2d[sl].reshape(dt_n, P, N_SLOTS)
        ids_c = np.ascontiguousarray(
            ids_c.transpose(1, 0, 2).reshape(P, dt_n * N_SLOTS).astype(np.float32)
        )
        in_maps.append({"xw": xw_c, "bb": bb_np, "ids": ids_c, "io": io_np})
    return in_maps


def _postprocess(results, value_match_ids, n_cores: int):
    b_core = B // n_cores
    dt_n = b_core // P
    nll_all = np.stack([np.asarray(r["nll"]) for r in results])  # [8, 128, dt*30]
    preds_all = np.stack([np.asarray(r["preds"]) for r in results])
    # [cores, P, dt, 30] -> [cores, dt, P, 30] -> [B, 30]
    nll_tok = (
        nll_all.reshape(n_cores, P, dt_n, N_SLOTS)
        .transpose(0, 2, 1, 3)
        .reshape(B, N_SLOTS)
    )
    preds_tok = (
        preds_all.reshape(n_cores, P, dt_n, N_SLOTS)
        .transpose(0, 2, 1, 3)
        .reshape(B, N_SLOTS)
    )

    ids2d = np.asarray(value_match_ids).reshape(B, N_SLOTS)
    valid = ids2d != -1
    count = int(valid.sum())
    if count > 0:
        loss = np.float32(
            np.where(valid, nll_tok, 0.0).sum(dtype=np.float64) / count
        )
    else:
        loss = np.float32(0.0)
    preds = np.where(valid, preds_tok, -1).reshape(-1).astype(np.int32)
    return loss, preds


def _run(inputs: dict, trace: bool = False):
    state_output = inputs["state_output"]
    value_match_ids = inputs["value_match_ids"]
    W = inputs["W"]
    b = inputs["b"]

    n_cores = N_CORES
    nc = _get_nc(B // n_cores)
    in_maps = _prep_inputs(state_output, value_match_ids, W, b, n_cores)
    res = run_bass_kernel_spmd(nc, in_maps, list(range(n_cores)), trace=trace)
    loss, preds = _postprocess(res.results, value_match_ids, n_cores)
    return (loss, preds), res


def kernel(**inputs) -> tuple:
    (loss, preds), _ = _run(inputs, trace=False)
    return loss, preds


if __name__ == "__main__":
    # Smoke test with random data
    rng = np.random.default_rng(0)
    inputs = {
        "state_output": rng.standard_normal((N_TOKENS, HIDDEN), dtype=np.float32),
        "op_ids": rng.integers(0, 3, size=(N_SLOTS,)),
        "value_match_ids": rng.integers(0, C, size=(N_TOKENS,)),
        "W": (rng.standard_normal((N_SLOTS, HIDDEN, C)) * 0.02).astype(np.float32),
        "b": (rng.standard_normal((N_SLOTS, C)) * 0.02).astype(np.float32),
    }
    loss, preds = kernel(**inputs)
    print("loss:", loss, "preds:", preds[:10])


# revision 29
# speedup vs baseline: 2.5373x; 2.5373x over previous
"""Trainium2 Bass kernel for slot-routed classifier head (moe_routing).

Reference computation (per token t, slot s = t % 30):
    logits = x[t] @ W[s] + b[s]            # [200]
    nll[t] = logsumexp(logits) - logits[id[t]]
    loss   = mean(nll over valid tokens);  preds[t] = argmax(logits)

Strategy:
  - Host: reorder X to slot-major transposed layout [30, 768, B_CORE] per core
    (data-parallel over the 4096 dialogs, 512 dialogs per core). This makes
    every matmul operand naturally laid out (contraction dim on partitions),
    so the device does zero transposes.
  - Device (per core): for each (slot, dialog-tile of 128):
      PE:  6 accumulating fp32 matmuls -> PSUM logits [128 tok, 200 cls]
      DVE: bias-add + PSUM evac (scalar_tensor_tensor)
           reduce_max -> m
           label gather: (iota == id) * logits, fused free-dim accum -> logit[id]
           argmax: (logits >= m) * (200 - iota), reduce_max -> 200 - argmax
      ACT: Exp with fused free-dim accum -> Z (no max-sub needed: |logits| < ~6)
    Tails: Ln(Z), nll = lnZ - label_logit, preds = 200 - mx; DMA out.
  - Host: mask invalid (-1) ids, fp64 sum for the scalar loss, gather preds.
"""

import os
from contextlib import ExitStack

import numpy as np

import concourse.bass as bass
import concourse.mybir as mybir
import concourse.tile as tile
from concourse.bass_utils import run_bass_kernel_spmd

# Problem constants (hardcoded per contract)
N_SLOTS = 30
HIDDEN = 768
C = 200  # num labels
N_TOKENS = 122880
N_CORES = 8
B = N_TOKENS // N_SLOTS  # 4096 dialogs
P = 128
KC = HIDDEN // P  # 6 contraction chunks

F32 = mybir.dt.float32
F16 = mybir.dt.float16
I32 = mybir.dt.int32
AX = mybir.AxisListType
OP = mybir.AluOpType
AF = mybir.ActivationFunctionType

# fp16 hi/lo split matmul: logits are computed at scale WSCALE (W pre-scaled
# on host to dodge fp16 subnormals); exp/label paths rescale by 1/WSCALE.
WSCALE = 64.0


def _build(b_core: int) -> bass.Bass:
    """Build the single-core SPMD program for a shard of b_core dialogs."""
    dt_n = b_core // P  # dialog tiles per core
    ncols = dt_n * N_SLOTS

    nc = bass.Bass("TRN2", target_bir_lowering=False, debug=False)

    # xw packs [Xhi | Xlo | W'hi | W'lo] (fp16) along the free dim:
    # one DMA (and one wait) per slot
    fw = 2 * b_core + 2 * C
    xw = nc.dram_tensor("xw", [N_SLOTS, HIDDEN, fw], F16, kind="ExternalInput").ap()
    bb = nc.dram_tensor("bb", [P, N_SLOTS, C], F32, kind="ExternalInput").ap()
    ids = nc.dram_tensor("ids", [P, ncols], F32, kind="ExternalInput").ap()
    io = nc.dram_tensor("io", [P, 2, C], F32, kind="ExternalInput").ap()
    nll = nc.dram_tensor("nll", [P, ncols], F32, kind="ExternalOutput").ap()
    preds = nc.dram_tensor("preds", [P, ncols], I32, kind="ExternalOutput").ap()

    with tile.TileContext(nc) as tc, ExitStack() as ctx:
        const_pool = ctx.enter_context(tc.tile_pool(name="const", bufs=1))
        xt_pool = ctx.enter_context(tc.tile_pool(name="xtp", bufs=3))
        psum_pool = ctx.enter_context(tc.tile_pool(name="ps", bufs=8, space="PSUM"))
        lg_pool = ctx.enter_context(tc.tile_pool(name="lg", bufs=8))
        e_pool = ctx.enter_context(tc.tile_pool(name="ep", bufs=4))
        j_pool = ctx.enter_context(tc.tile_pool(name="jp", bufs=8))

        # Constants / accumulators (persistent)
        bb_sb = const_pool.tile([P, N_SLOTS, C], F32)
        nc.sync.dma_start(bb_sb[:], bb[:])
        ids_sb = const_pool.tile([P, ncols], F32)
        nc.sync.dma_start(ids_sb[:], ids[:])
        io_sb = const_pool.tile([P, 2, C], F32)
        nc.sync.dma_start(io_sb[:], io[:])

        # One-time wait absorbers: the S2S2D2_STT struct encodes only one sem
        # wait, so pre-touch each const DMA on DVE; later DVE ops then only
        # ever wait on the PE semaphore.
        warm = const_pool.tile([P, 4], F32)
        nc.vector.tensor_copy(warm[:, 0:1], bb_sb[:, 0, 0:1])
        nc.vector.tensor_copy(warm[:, 1:2], ids_sb[:, 0:1])
        nc.vector.tensor_copy(warm[:, 2:3], io_sb[:, 0, 0:1])

        m_all = const_pool.tile([P, ncols], F32)
        z_all = const_pool.tile([P, ncols], F32)
        lbl_all = const_pool.tile([P, ncols], F32)
        mx_all = const_pool.tile([P, ncols], F32)

        o_xlo = b_core
        o_whi = 2 * b_core
        o_wlo = 2 * b_core + C
        for s in range(N_SLOTS):
            xw_sb = xt_pool.tile([P, KC, fw], F16)
            nc.sync.dma_start(xw_sb[:], xw[s].rearrange("(k p) f -> p k f", p=P))

            for dti in range(dt_n):
                col = dti * N_SLOTS + s
                ps = psum_pool.tile([P, C], F32)
                for k in range(KC):
                    xhi = xw_sb[:, k, bass.ts(dti, P)]
                    xlo = xw_sb[:, k, o_xlo + dti * P : o_xlo + (dti + 1) * P]
                    whi = xw_sb[:, k, o_whi : o_whi + C]
                    wlo = xw_sb[:, k, o_wlo : o_wlo + C]
                    # logits*WSCALE = Xhi@W'hi + Xhi@W'lo + Xlo@W'hi
                    nc.tensor.matmul(ps[:], xhi, whi, start=(k == 0), stop=False)
                    nc.tensor.matmul(ps[:], xhi, wlo, start=False, stop=False)
                    nc.tensor.matmul(
                        ps[:], xlo, whi, start=False, stop=(k == KC - 1)
                    )
                # logits = psum + b_s   (bias add + PSUM evacuation in one op)
                lg = lg_pool.tile([P, C], F32)
                nc.vector.scalar_tensor_tensor(
                    out=lg[:], in0=ps[:], scalar=0.0, op0=OP.bypass,
                    in1=bb_sb[:, s, :], op1=OP.add,
                )
                # row max (for argmax)
                nc.vector.reduce_max(m_all[:, col : col + 1], lg[:], axis=AX.X)
                # e = exp(logits); Z = sum(e) fused
                ej = e_pool.tile([P, C], F32)
                nc.scalar.activation(
                    ej[:], lg[:], AF.Exp, scale=1.0 / WSCALE,
                    accum_out=z_all[:, col : col + 1],
                )
                # label logit: sum((iota == id) * logits)
                j1 = j_pool.tile([P, C], F32)
                nc.vector.scalar_tensor_tensor(
                    out=j1[:], in0=io_sb[:, 0, :], scalar=ids_sb[:, col : col + 1],
                    op0=OP.is_equal, in1=lg[:], op1=OP.mult,
                    accum_out=lbl_all[:, col : col + 1],
                )
                # argmax: max((logits >= m) * (200 - iota)) = 200 - argmax
                j2 = j_pool.tile([P, C], F32)
                nc.vector.scalar_tensor_tensor(
                    out=j2[:], in0=lg[:], scalar=m_all[:, col : col + 1],
                    op0=OP.is_ge, in1=io_sb[:, 1, :], op1=OP.mult,
                )
                nc.vector.reduce_max(mx_all[:, col : col + 1], j2[:], axis=AX.X)

        # Tails  (lbl_all is at scale WSCALE; nll = lnZ - lbl/WSCALE)
        lnz = const_pool.tile([P, ncols], F32)
        nc.scalar.activation(lnz[:], z_all[:], AF.Ln)
        nll_sb = const_pool.tile([P, ncols], F32)
        nc.vector.scalar_tensor_tensor(
            out=nll_sb[:], in0=lbl_all[:], scalar=-1.0 / WSCALE, op0=OP.mult,
            in1=lnz[:], op1=OP.add,
        )
        preds_sb = const_pool.tile([P, ncols], I32)
        nc.vector.tensor_scalar(
            preds_sb[:], mx_all[:], -1.0, 200.0, op0=OP.mult, op1=OP.add
        )
        nc.sync.dma_start(nll[:], nll_sb[:])
        nc.sync.dma_start(preds[:], preds_sb[:])

    _legalize_waits(nc)
    _split_residual_waits_in_json(nc)
    return nc


# ISA structs for compute ops encode a single sem-wait command; Tile's
# scheduler freely attaches several. Legalize: (1) drop waits on an engine's
# own sem that program order already satisfies, (2) hoist extra waits onto an
# earlier same-engine instruction with a free wait slot (safe: engines are
# in-order, so waiting earlier only strengthens the schedule).
_ONE_WAIT_OPS = {
    "Matmult", "Activation", "TensorScalarPtr", "TensorReduce", "TensorTensor",
    "TensorCopy", "TensorScalar", "Memset", "Iota", "TensorMaskReduce",
    "DMACopy",
}


def _legalize_waits(nc: bass.Bass, one_wait_ops=_ONE_WAIT_OPS):
    import bass_rust
    for f in nc.m.functions:
        for bl in f.blocks:
            insns = list(bl.instructions)
            sem_updaters: dict[int, set] = {}
            sem_async: set[int] = set()  # sems inc'd at async DMA completion
            for ins in insns:
                si = ins.sync_info
                if si:
                    for u in si.on_update:
                        sem_updaters.setdefault(u.id, set()).add(ins.engine)
                        if ins.opcode == "DMACopy":
                            sem_async.add(u.id)

            # sems incremented exclusively by one engine's (sync) instructions
            sync_engine_sem: dict[int, object] = {}
            for sid, ups in sem_updaters.items():
                if len(ups) == 1 and sid not in sem_async:
                    sync_engine_sem[sid] = next(iter(ups))

            # Pass A: implication tables. For each sync-engine sem S, for each
            # inc index n, the max (other_sem -> wait_value) seen on S's
            # engine stream before the n-th inc completes.
            # impl[S] = list of (inc_index, {sem: maxval}) snapshots.
            running: dict[object, dict[int, int]] = {}
            impl: dict[int, list] = {}
            inc_count: dict[int, int] = {}
            for ins in insns:
                eng = ins.engine
                si = ins.sync_info
                if not si:
                    continue
                r = running.setdefault(eng, {})
                for w in si.on_wait:
                    if w.wait_mode == "sem-ge-imm":
                        if r.get(w.id, -1) < w.wait_value:
                            r[w.id] = w.wait_value
                for u in si.on_update:
                    if u.update_mode == "sem-inc" and sync_engine_sem.get(u.id) == eng:
                        inc_count[u.id] = inc_count.get(u.id, 0) + u.update_value
                        impl.setdefault(u.id, []).append(
                            (inc_count[u.id], dict(r))
                        )

            def implied(kept_waits, w):
                """Is wait w implied by any wait already kept?"""
                for k in kept_waits:
                    tab = impl.get(k.id)
                    if not tab or k.wait_mode != "sem-ge-imm":
                        continue
                    # snapshot at the largest inc index <= k.wait_value
                    snap = None
                    for n, d in tab:
                        if n <= k.wait_value:
                            snap = d
                        else:
                            break
                    if snap is not None and snap.get(w.id, -1) >= w.wait_value:
                        return True
                return False

            # Pass B: rewrite
            inc_seen: dict[int, int] = {}
            targets: dict = {}
            unplaced = 0
            for ins in insns:
                eng = ins.engine
                si = ins.sync_info
                waits = list(si.on_wait) if si else []
                updates = list(si.on_update) if si else []
                if ins.opcode in one_wait_ops and len(waits) > 1:
                    kept = []
                    for w in waits:
                        if (
                            sync_engine_sem.get(w.id) == eng
                            and w.wait_mode == "sem-ge-imm"
                            and inc_seen.get(w.id, 0) >= w.wait_value
                        ):
                            continue  # program order satisfies it
                        kept.append(w)
                    if len(kept) > 1:
                        primary = kept[:1]
                        for w in kept[1:]:
                            if not implied(primary, w):
                                primary.append(w)
                        kept = primary
                    # leftovers with >1 wait are split into preceding Drains
                    # at the JSON level (_split_residual_waits_in_json)
                    ins.sync_info = bass_rust.SyncInfo(on_wait=kept, on_update=updates)
                for u in updates:
                    if u.update_mode == "sem-inc":
                        inc_seen[u.id] = inc_seen.get(u.id, 0) + u.update_value
                if ins.opcode not in ("UnconditionalBranch", "Call"):
                    cur = ins.sync_info
                    if ins.opcode == "Drain" or not cur or len(cur.on_wait) == 0:
                        targets.setdefault(eng, []).append(ins)
                        if len(targets[eng]) > 64:
                            targets[eng] = targets[eng][-64:]
            if unplaced:
                import logging
                logging.warning(f"legalize_waits: {unplaced} waits could not be placed")


def _split_residual_waits_in_json(nc: bass.Bass):
    """Final walrus-facing fix: split any instruction still carrying >1 sem
    wait into preceding single-wait Drains on the same engine (pure wait
    sequencing — no reordering), and pin the patched JSON onto the instance
    so every downstream serialization (bass2jax, compile) uses it."""
    import orjson

    bir = orjson.loads(type(nc).to_json_bytes(nc))
    n = 0
    for f in bir["functions"]:
        for bl in f["blocks"]:
            out = []
            for ins in bl.get("instructions", []):
                si = ins.get("sync_info") or {}
                waits = si.get("on_wait") or []
                if len(waits) > 1:
                    for w in waits[:-1]:
                        n += 1
                        out.append({
                            "debug": ins.get("debug", 0),
                            "engine": ins["engine"],
                            "ins": [],
                            "outs": [],
                            "name": f"{ins['name']}-lgw{n}",
                            "opcode": "Drain",
                            "sync_info": {"on_update": [], "on_wait": [w]},
                        })
                    si = dict(si)
                    si["on_wait"] = waits[-1:]
                    ins = dict(ins)
                    ins["sync_info"] = si
                out.append(ins)
            bl["instructions"] = out
    data = orjson.dumps(bir)
    nc.to_json_bytes = lambda: data  # type: ignore[method-assign]


_NC_CACHE: dict[int, bass.Bass] = {}


def _get_nc(b_core: int) -> bass.Bass:
    if b_core not in _NC_CACHE:
        _NC_CACHE[b_core] = _build(b_core)
    return _NC_CACHE[b_core]


def _prep_inputs(state_output, value_match_ids, W, b, n_cores: int):
    """Host-side sharding: slot-major transposed X, per-core in_maps."""
    b_core = B // n_cores
    dt_n = b_core // P
    x3 = np.ascontiguousarray(state_output, dtype=np.float32).reshape(B, N_SLOTS, HIDDEN)
    # [30, 768, 4096]: xt[s, d, dialog]
    xt_full = x3.transpose(1, 2, 0)
    xhi_full = xt_full.astype(np.float16)
    xlo_full = (xt_full - xhi_full.astype(np.float32)).astype(np.float16)

    ws = np.asarray(W, dtype=np.float32) * np.float32(WSCALE)
    whi = ws.astype(np.float16)
    wlo = (ws - whi.astype(np.float32)).astype(np.float16)

    bb_np = np.ascontiguousarray(
        np.broadcast_to(
            np.asarray(b, dtype=np.float32) * np.float32(WSCALE), (P, N_SLOTS, C)
        )
    )
    iota = np.arange(C, dtype=np.float32)
    io_np = np.ascontiguousarray(
        np.broadcast_to(np.stack([iota, C - iota]), (P, 2, C))
    )

    ids2d = np.asarray(value_match_ids).reshape(B, N_SLOTS)
    in_maps = []
    for c in range(n_cores):
        sl = slice(c * b_core, (c + 1) * b_core)
        xw_c = np.empty((N_SLOTS, HIDDEN, 2 * b_core + 2 * C), dtype=np.float16)
        xw_c[:, :, :b_core] = xhi_full[:, :, sl]
        xw_c[:, :, b_core : 2 * b_core] = xlo_full[:, :, sl]
        xw_c[:, :, 2 * b_core : 2 * b_core + C] = whi
        xw_c[:, :, 2 * b_core + C :] = wlo
        ids_c = ids2d[sl].reshape(dt_n, P, N_SLOTS)
        ids_c = np.ascontiguousarray(
            ids_c.transpose(1, 0, 2).reshape(P, dt_n * N_SLOTS).astype(np.float32)
        )
        in_maps.append({"xw": xw_c, "bb": bb_np, "ids": ids_c, "io": io_np})
    return in_maps


def _postprocess(results, value_match_ids, n_cores: int):
    b_core = B // n_cores
    dt_n = b_core // P
    nll_all = np.stack([np.asarray(r["nll"]) for r in results])  # [8, 128, dt*30]
    preds_all = np.stack([np.asarray(r["preds"]) for r in results])
    # [cores, P, dt, 30] -> [cores, dt, P, 30] -> [B, 30]
    nll_tok = (
        nll_all.reshape(n_cores, P, dt_n, N_SLOTS)
        .transpose(0, 2, 1, 3)
        .reshape(B, N_SLOTS)
    )
    preds_tok = (
        preds_all.reshape(n_cores, P, dt_n, N_SLOTS)
        .transpose(0, 2, 1, 3)
        .reshape(B, N_SLOTS)
    )

    ids2d = np.asarray(value_match_ids).reshape(B, N_SLOTS)
    valid = ids2d != -1
    count = int(valid.sum())
    if count > 0:
        loss = np.float32(
            np.where(valid, nll_tok, 0.0).sum(dtype=np.float64) / count
        )
    else:
        loss = np.float32(0.0)
    preds = np.where(valid, preds_tok, -1).reshape(-1).astype(np.int32)
    return loss, preds


def _run(inputs: dict, trace: bool = False):
    state_output = inputs["state_output"]
    value_match_ids = inputs["value_match_ids"]
    W = inputs["W"]
    b = inputs["b"]

    n_cores = N_CORES
    nc = _get_nc(B // n_cores)
    in_maps = _prep_inputs(state_output, value_match_ids, W, b, n_cores)
    res = run_bass_kernel_spmd(nc, in_maps, list(range(n_cores)), trace=trace)
    loss, preds = _postprocess(res.results, value_match_ids, n_cores)
    return (loss, preds), res


def kernel(**inputs) -> tuple:
    (loss, preds), _ = _run(inputs, trace=False)
    return loss, preds


if __name__ == "__main__":
    # Smoke test with random data
    rng = np.random.default_rng(0)
    inputs = {
        "state_output": rng.standard_normal((N_TOKENS, HIDDEN), dtype=np.float32),
        "op_ids": rng.integers(0, 3, size=(N_SLOTS,)),
        "value_match_ids": rng.integers(0, C, size=(N_TOKENS,)),
        "W": (rng.standard_normal((N_SLOTS, HIDDEN, C)) * 0.02).astype(np.float32),
        "b": (rng.standard_normal((N_SLOTS, C)) * 0.02).astype(np.float32),
    }
    loss, preds = kernel(**inputs)
    print("loss:", loss, "preds:", preds[:10])


# revision 38
# speedup vs baseline: 2.9725x; 1.1715x over previous
"""Trainium2 Bass kernel for slot-routed classifier head (moe_routing).

Reference computation (per token t, slot s = t % 30):
    logits = x[t] @ W[s] + b[s]            # [200]
    nll[t] = logsumexp(logits) - logits[id[t]]
    loss   = mean(nll over valid tokens);  preds[t] = argmax(logits)

Strategy:
  - Host: reorder X to slot-major transposed layout [30, 768, B_CORE] per core
    (data-parallel over the 4096 dialogs, 512 dialogs per core). This makes
    every matmul operand naturally laid out (contraction dim on partitions),
    so the device does zero transposes.
  - Device (per core): for each (slot, dialog-tile of 128):
      PE:  6 accumulating fp32 matmuls -> PSUM logits [128 tok, 200 cls]
      DVE: bias-add + PSUM evac (scalar_tensor_tensor)
           reduce_max -> m
           label gather: (iota == id) * logits, fused free-dim accum -> logit[id]
           argmax: (logits >= m) * (200 - iota), reduce_max -> 200 - argmax
      ACT: Exp with fused free-dim accum -> Z (no max-sub needed: |logits| < ~6)
    Tails: Ln(Z), nll = lnZ - label_logit, preds = 200 - mx; DMA out.
  - Host: mask invalid (-1) ids, fp64 sum for the scalar loss, gather preds.
"""

import os
from contextlib import ExitStack

import numpy as np

import concourse.bass as bass
import concourse.bacc as bacc
import concourse.mybir as mybir
import concourse.tile as tile
from concourse.bass_utils import run_bass_kernel_spmd

# Problem constants (hardcoded per contract)
N_SLOTS = 30
HIDDEN = 768
C = 200  # num labels
N_TOKENS = 122880
N_CORES = 8
B = N_TOKENS // N_SLOTS  # 4096 dialogs
P = 128
KC = HIDDEN // P  # 6 contraction chunks

F32 = mybir.dt.float32
F16 = mybir.dt.float16
I32 = mybir.dt.int32
AX = mybir.AxisListType
OP = mybir.AluOpType
AF = mybir.ActivationFunctionType

# fp16 hi/lo split matmul: logits are computed at scale WSCALE (W pre-scaled
# on host to dodge fp16 subnormals); exp/label paths rescale by 1/WSCALE.
WSCALE = 64.0


def _register_dve_ops():
    """Register three fused custom DVE ops (runtime registration — the table
    is generated per-NEFF at compile time, no firmware change needed):
      ANT_ADD_MAXRED: out = in0 + in1; accum = max(out)   (bias+evac+rowmax)
      ANT_LABEL_SUM:  out = (Idx == s0) ? in0 : 0; accum = sum  (label gather)
      ANT_ARGMAX_REV: out = (in0 >= s0) ? imm2 - Idx : 0; accum = max
                      (=> imm2 - first argmax index)
    """
    from operator import add as _add

    import concourse.dve_ops as dve_ops
    from concourse.dve_spec import (
        Spec, Src0, C0, C2, Zero, Idx, select, eq, maxx, lower,
    )
    from concourse.dve_spec import Src1
    from concourse.dve_ops import has_src1
    from concourse.dve_uop import DveOpSpec

    def _ref_add_maxred(in0, in1, c0, c1, c2):
        out = in0.astype(np.float32) + in1.astype(np.float32)
        acc = out.reshape(out.shape[0], -1).max(axis=-1, keepdims=True)
        return out, acc

    def _ref_label_sum(in0, in1, c0, c1, c2):
        f = in0.astype(np.float32).reshape(in0.shape[0], -1)
        idx = np.arange(f.shape[1], dtype=np.float32)[None, :]
        c0a = np.asarray(c0, dtype=np.float32).reshape(-1, 1)
        out = np.where(idx == c0a, f, np.float32(0.0))
        return out.reshape(in0.shape), out.sum(axis=-1, keepdims=True)

    def _ref_argmax_rev(in0, in1, c0, c1, c2):
        f = in0.astype(np.float32).reshape(in0.shape[0], -1)
        idx = np.arange(f.shape[1], dtype=np.float32)[None, :]
        c0a = np.asarray(c0, dtype=np.float32).reshape(-1, 1)
        out = np.where(f >= c0a, np.float32(c2) - idx, np.float32(0.0))
        return out.reshape(in0.shape), out.max(axis=-1, keepdims=True)

    specs = [
        ("ANT_ADD_MAXRED",
         Spec(body=Src0 + Src1, accum=maxx, reference=_ref_add_maxred)),
        ("ANT_LABEL_SUM",
         Spec(body=select(eq(Idx, C0), Src0, Zero), accum=_add,
              reference=_ref_label_sum)),
        ("ANT_ARGMAX_REV",
         Spec(body=select(Src0 >= C0, C2 - Idx, Zero), accum=maxx,
              reference=_ref_argmax_rev)),
    ]
    out = {}
    for name, spec in specs:
        if name in dve_ops._SUB_OPCODE_FOR_NAME:
            out[name] = next(o for o in dve_ops.OPS if o.name == name)
            continue
        opcode = dve_ops._CUSTOM_DVE_ROW_BASE + len(dve_ops.OPS)
        shas = {}
        for ver in ("v3", "v4"):
            try:
                u = lower(spec, ver=ver)
                shas[ver] = DveOpSpec(
                    name=name, opcode=opcode, uops=u, rd1_en=has_src1(spec)
                ).sha(ver)
            except Exception:
                pass
        op = dve_ops.DveOp(name, spec, subdim=False, uops_sha=shas)
        dve_ops.OPS.append(op)
        dve_ops._SUB_OPCODE_FOR_NAME[name] = opcode
        dve_ops.CUSTOM_DVE_SPECS[name] = spec
        out[name] = op
    return out


_DVE_OPS = _register_dve_ops()


def _build(b_core: int) -> bass.Bass:
    """Build the single-core SPMD program for a shard of b_core dialogs."""
    dt_n = b_core // P  # dialog tiles per core
    ncols = dt_n * N_SLOTS

    nc = bacc.Bacc("TRN2", target_bir_lowering=False, debug=False)

    # xw packs [Xhi | Xlo | W'hi | W'lo] (fp16) along the free dim:
    # one DMA (and one wait) per slot
    fw = 2 * b_core + 2 * C
    xw = nc.dram_tensor("xw", [N_SLOTS, HIDDEN, fw], F16, kind="ExternalInput").ap()
    bb = nc.dram_tensor("bb", [P, N_SLOTS, C], F32, kind="ExternalInput").ap()
    ids = nc.dram_tensor("ids", [P, ncols], F32, kind="ExternalInput").ap()
    nll = nc.dram_tensor("nll", [P, ncols], F32, kind="ExternalOutput").ap()
    preds = nc.dram_tensor("preds", [P, ncols], I32, kind="ExternalOutput").ap()

    with tile.TileContext(nc) as tc, ExitStack() as ctx:
        const_pool = ctx.enter_context(tc.tile_pool(name="const", bufs=1))
        xt_pool = ctx.enter_context(tc.tile_pool(name="xtp", bufs=3))
        psum_pool = ctx.enter_context(tc.tile_pool(name="ps", bufs=8, space="PSUM"))
        lg_pool = ctx.enter_context(tc.tile_pool(name="lg", bufs=8))
        e_pool = ctx.enter_context(tc.tile_pool(name="ep", bufs=4))
        j_pool = ctx.enter_context(tc.tile_pool(name="jp", bufs=8))

        # Constants / accumulators (persistent)
        bb_sb = const_pool.tile([P, N_SLOTS, C], F32)
        nc.sync.dma_start(bb_sb[:], bb[:])
        ids_sb = const_pool.tile([P, ncols], F32)
        nc.sync.dma_start(ids_sb[:], ids[:])

        # One-time wait absorbers: compute-op ISA structs encode only one sem
        # wait, so pre-touch each const DMA on DVE; later DVE ops then only
        # ever wait on the PE semaphore.
        warm = const_pool.tile([P, 4], F32)
        nc.vector.tensor_copy(warm[:, 0:1], bb_sb[:, 0, 0:1])
        nc.vector.tensor_copy(warm[:, 1:2], ids_sb[:, 0:1])

        m_all = const_pool.tile([P, ncols], F32)
        z_all = const_pool.tile([P, ncols], F32)
        lbl_all = const_pool.tile([P, ncols], F32)
        mx_all = const_pool.tile([P, ncols], F32)

        o_xlo = b_core
        o_whi = 2 * b_core
        o_wlo = 2 * b_core + C
        for s in range(N_SLOTS):
            xw_sb = xt_pool.tile([P, KC, fw], F16)
            nc.sync.dma_start(xw_sb[:], xw[s].rearrange("(k p) f -> p k f", p=P))

            for dti in range(dt_n):
                col = dti * N_SLOTS + s
                ps = psum_pool.tile([P, C], F32)
                for k in range(KC):
                    xhi = xw_sb[:, k, bass.ts(dti, P)]
                    xlo = xw_sb[:, k, o_xlo + dti * P : o_xlo + (dti + 1) * P]
                    whi = xw_sb[:, k, o_whi : o_whi + C]
                    wlo = xw_sb[:, k, o_wlo : o_wlo + C]
                    # logits*WSCALE = Xhi@W'hi + Xhi@W'lo + Xlo@W'hi
                    nc.tensor.matmul(ps[:], xhi, whi, start=(k == 0), stop=False)
                    nc.tensor.matmul(ps[:], xhi, wlo, start=False, stop=False)
                    nc.tensor.matmul(
                        ps[:], xlo, whi, start=False, stop=(k == KC - 1)
                    )
                # logits = psum + b_s; m = rowmax  (bias+evac+max in ONE op)
                lg = lg_pool.tile([P, C], F32)
                nc.vector._custom_dve(
                    _DVE_OPS["ANT_ADD_MAXRED"], out=lg[:], in0=ps[:],
                    in1=bb_sb[:, s, :], accum_out=m_all[:, col : col + 1],
                )
                # e = exp(logits); Z = sum(e) fused
                ej = e_pool.tile([P, C], F32)
                nc.scalar.activation(
                    ej[:], lg[:], AF.Exp, scale=1.0 / WSCALE,
                    accum_out=z_all[:, col : col + 1],
                )
                # label logit: sum((Idx == id) * logits) in ONE op
                j1 = j_pool.tile([P, C], F32)
                nc.vector._custom_dve(
                    _DVE_OPS["ANT_LABEL_SUM"], out=j1[:], in0=lg[:],
                    s0=ids_sb[:, col : col + 1],
                    accum_out=lbl_all[:, col : col + 1],
                )
                # argmax: max((logits >= m) ? 200 - Idx : 0) = 200 - argmax
                j2 = j_pool.tile([P, C], F32)
                nc.vector._custom_dve(
                    _DVE_OPS["ANT_ARGMAX_REV"], out=j2[:], in0=lg[:],
                    s0=m_all[:, col : col + 1], imm2=float(C),
                    accum_out=mx_all[:, col : col + 1],
                )

        # Tails  (lbl_all is at scale WSCALE; nll = lnZ - lbl/WSCALE)
        lnz = const_pool.tile([P, ncols], F32)
        nc.scalar.activation(lnz[:], z_all[:], AF.Ln)
        nll_sb = const_pool.tile([P, ncols], F32)
        nc.vector.scalar_tensor_tensor(
            out=nll_sb[:], in0=lbl_all[:], scalar=-1.0 / WSCALE, op0=OP.mult,
            in1=lnz[:], op1=OP.add,
        )
        preds_sb = const_pool.tile([P, ncols], I32)
        nc.vector.tensor_scalar(
            preds_sb[:], mx_all[:], -1.0, 200.0, op0=OP.mult, op1=OP.add
        )
        nc.sync.dma_start(nll[:], nll_sb[:])
        nc.sync.dma_start(preds[:], preds_sb[:])

    _legalize_waits(nc)  # drop provably-redundant waits (fewer event-sems)
    nc.finalize()  # Bacc pipeline: wait splitting via event sems + ISA codegen
    return nc


# ISA structs for compute ops encode a single sem-wait command; Tile's
# scheduler freely attaches several. Legalize: (1) drop waits on an engine's
# own sem that program order already satisfies, (2) hoist extra waits onto an
# earlier same-engine instruction with a free wait slot (safe: engines are
# in-order, so waiting earlier only strengthens the schedule).
_ONE_WAIT_OPS = {
    "Matmult", "Activation", "TensorScalarPtr", "TensorReduce", "TensorTensor",
    "TensorCopy", "TensorScalar", "Memset", "Iota", "TensorMaskReduce",
    "DMACopy", "CustomDveAnt", "ISA",
}


def _legalize_waits(nc: bass.Bass, one_wait_ops=_ONE_WAIT_OPS):
    import bass_rust
    for f in nc.m.functions:
        for bl in f.blocks:
            insns = list(bl.instructions)
            sem_updaters: dict[int, set] = {}
            sem_async: set[int] = set()  # sems inc'd at async DMA completion
            for ins in insns:
                si = ins.sync_info
                if si:
                    for u in si.on_update:
                        sem_updaters.setdefault(u.id, set()).add(ins.engine)
                        if ins.opcode == "DMACopy":
                            sem_async.add(u.id)

            # sems incremented exclusively by one engine's (sync) instructions
            sync_engine_sem: dict[int, object] = {}
            for sid, ups in sem_updaters.items():
                if len(ups) == 1 and sid not in sem_async:
                    sync_engine_sem[sid] = next(iter(ups))

            # Pass A: implication tables. For each sync-engine sem S, for each
            # inc index n, the max (other_sem -> wait_value) seen on S's
            # engine stream before the n-th inc completes.
            # impl[S] = list of (inc_index, {sem: maxval}) snapshots.
            running: dict[object, dict[int, int]] = {}
            impl: dict[int, list] = {}
            inc_count: dict[int, int] = {}
            for ins in insns:
                eng = ins.engine
                si = ins.sync_info
                if not si:
                    continue
                r = running.setdefault(eng, {})
                for w in si.on_wait:
                    if w.wait_mode == "sem-ge-imm":
                        if r.get(w.id, -1) < w.wait_value:
                            r[w.id] = w.wait_value
                for u in si.on_update:
                    if u.update_mode == "sem-inc" and sync_engine_sem.get(u.id) == eng:
                        inc_count[u.id] = inc_count.get(u.id, 0) + u.update_value
                        impl.setdefault(u.id, []).append(
                            (inc_count[u.id], dict(r))
                        )

            def implied(kept_waits, w):
                """Is wait w implied by any wait already kept?"""
                for k in kept_waits:
                    tab = impl.get(k.id)
                    if not tab or k.wait_mode != "sem-ge-imm":
                        continue
                    # snapshot at the largest inc index <= k.wait_value
                    snap = None
                    for n, d in tab:
                        if n <= k.wait_value:
                            snap = d
                        else:
                            break
                    if snap is not None and snap.get(w.id, -1) >= w.wait_value:
                        return True
                return False

            # Pass B: rewrite
            inc_seen: dict[int, int] = {}
            targets: dict = {}
            unplaced = 0
            for ins in insns:
                eng = ins.engine
                si = ins.sync_info
                waits = list(si.on_wait) if si else []
                updates = list(si.on_update) if si else []
                if ins.opcode in one_wait_ops and len(waits) > 1:
                    kept = []
                    for w in waits:
                        if (
                            sync_engine_sem.get(w.id) == eng
                            and w.wait_mode == "sem-ge-imm"
                            and inc_seen.get(w.id, 0) >= w.wait_value
                        ):
                            continue  # program order satisfies it
                        kept.append(w)
                    if len(kept) > 1:
                        primary = kept[:1]
                        for w in kept[1:]:
                            if not implied(primary, w):
                                primary.append(w)
                        kept = primary
                    # leftovers with >1 wait are split into preceding Drains
                    # at the JSON level (_split_residual_waits_in_json)
                    ins.sync_info = bass_rust.SyncInfo(on_wait=kept, on_update=updates)
                for u in updates:
                    if u.update_mode == "sem-inc":
                        inc_seen[u.id] = inc_seen.get(u.id, 0) + u.update_value
                if ins.opcode not in ("UnconditionalBranch", "Call"):
                    cur = ins.sync_info
                    if ins.opcode == "Drain" or not cur or len(cur.on_wait) == 0:
                        targets.setdefault(eng, []).append(ins)
                        if len(targets[eng]) > 64:
                            targets[eng] = targets[eng][-64:]
            if unplaced:
                import logging
                logging.warning(f"legalize_waits: {unplaced} waits could not be placed")


def _split_residual_waits_in_json(nc: bass.Bass):
    """Final walrus-facing fix: split any instruction still carrying >1 sem
    wait into preceding single-wait Drains on the same engine (pure wait
    sequencing — no reordering), and pin the patched JSON onto the instance
    so every downstream serialization (bass2jax, compile) uses it."""
    import orjson

    bir = orjson.loads(type(nc).to_json_bytes(nc))
    n = 0
    for f in bir["functions"]:
        for bl in f["blocks"]:
            out = []
            for ins in bl.get("instructions", []):
                si = ins.get("sync_info") or {}
                waits = si.get("on_wait") or []
                # raw-ISA instructions (custom DVE) cannot carry any inline
                # sem wait ("ISA wrong length"); others carry exactly one.
                keep = 0 if ins.get("opcode") == "ISA" else 1
                if len(waits) > keep:
                    for w in waits[: len(waits) - keep]:
                        n += 1
                        out.append({
                            "debug": ins.get("debug", 0),
                            "engine": ins["engine"],
                            "ins": [],
                            "outs": [],
                            "name": f"{ins['name']}-lgw{n}",
                            "opcode": "Drain",
                            "sync_info": {"on_update": [], "on_wait": [w]},
                        })
                    si = dict(si)
                    si["on_wait"] = waits[len(waits) - keep :]
                    ins = dict(ins)
                    ins["sync_info"] = si
                out.append(ins)
            bl["instructions"] = out
    data = orjson.dumps(bir)
    nc.to_json_bytes = lambda: data  # type: ignore[method-assign]


_NC_CACHE: dict[int, bass.Bass] = {}


def _get_nc(b_core: int) -> bass.Bass:
    if b_core not in _NC_CACHE:
        _NC_CACHE[b_core] = _build(b_core)
    return _NC_CACHE[b_core]


def _prep_inputs(state_output, value_match_ids, W, b, n_cores: int):
    """Host-side sharding: slot-major transposed X, per-core in_maps."""
    b_core = B // n_cores
    dt_n = b_core // P
    x3 = np.ascontiguousarray(state_output, dtype=np.float32).reshape(B, N_SLOTS, HIDDEN)
    # [30, 768, 4096]: xt[s, d, dialog]
    xt_full = x3.transpose(1, 2, 0)
    xhi_full = xt_full.astype(np.float16)
    xlo_full = (xt_full - xhi_full.astype(np.float32)).astype(np.float16)

    ws = np.asarray(W, dtype=np.float32) * np.float32(WSCALE)
    whi = ws.astype(np.float16)
    wlo = (ws - whi.astype(np.float32)).astype(np.float16)

    bb_np = np.ascontiguousarray(
        np.broadcast_to(
            np.asarray(b, dtype=np.float32) * np.float32(WSCALE), (P, N_SLOTS, C)
        )
    )
    ids2d = np.asarray(value_match_ids).reshape(B, N_SLOTS)
    in_maps = []
    for c in range(n_cores):
        sl = slice(c * b_core, (c + 1) * b_core)
        xw_c = np.empty((N_SLOTS, HIDDEN, 2 * b_core + 2 * C), dtype=np.float16)
        xw_c[:, :, :b_core] = xhi_full[:, :, sl]
        xw_c[:, :, b_core : 2 * b_core] = xlo_full[:, :, sl]
        xw_c[:, :, 2 * b_core : 2 * b_core + C] = whi
        xw_c[:, :, 2 * b_core + C :] = wlo
        ids_c = ids2d[sl].reshape(dt_n, P, N_SLOTS)
        ids_c = np.ascontiguousarray(
            ids_c.transpose(1, 0, 2).reshape(P, dt_n * N_SLOTS).astype(np.float32)
        )
        in_maps.append({"xw": xw_c, "bb": bb_np, "ids": ids_c})
    return in_maps


def _postprocess(results, value_match_ids, n_cores: int):
    b_core = B // n_cores
    dt_n = b_core // P
    nll_all = np.stack([np.asarray(r["nll"]) for r in results])  # [8, 128, dt*30]
    preds_all = np.stack([np.asarray(r["preds"]) for r in results])
    # [cores, P, dt, 30] -> [cores, dt, P, 30] -> [B, 30]
    nll_tok = (
        nll_all.reshape(n_cores, P, dt_n, N_SLOTS)
        .transpose(0, 2, 1, 3)
        .reshape(B, N_SLOTS)
    )
    preds_tok = (
        preds_all.reshape(n_cores, P, dt_n, N_SLOTS)
        .transpose(0, 2, 1, 3)
        .reshape(B, N_SLOTS)
    )

    ids2d = np.asarray(value_match_ids).reshape(B, N_SLOTS)
    valid = ids2d != -1
    count = int(valid.sum())
    if count > 0:
        loss = np.float32(
            np.where(valid, nll_tok, 0.0).sum(dtype=np.float64) / count
        )
    else:
        loss = np.float32(0.0)
    preds = np.where(valid, preds_tok, -1).reshape(-1).astype(np.int32)
    return loss, preds


def _run(inputs: dict, trace: bool = False):
    state_output = inputs["state_output"]
    value_match_ids = inputs["value_match_ids"]
    W = inputs["W"]
    b = inputs["b"]

    n_cores = N_CORES
    nc = _get_nc(B // n_cores)
    in_maps = _prep_inputs(state_output, value_match_ids, W, b, n_cores)
    res = run_bass_kernel_spmd(nc, in_maps, list(range(n_cores)), trace=trace)
    loss, preds = _postprocess(res.results, value_match_ids, n_cores)
    return (loss, preds), res


def kernel(**inputs) -> tuple:
    (loss, preds), _ = _run(inputs, trace=False)
    return loss, preds


if __name__ == "__main__":
    # Smoke test with random data
    rng = np.random.default_rng(0)
    inputs = {
        "state_output": rng.standard_normal((N_TOKENS, HIDDEN), dtype=np.float32),
        "op_ids": rng.integers(0, 3, size=(N_SLOTS,)),
        "value_match_ids": rng.integers(0, C, size=(N_TOKENS,)),
        "W": (rng.standard_normal((N_SLOTS, HIDDEN, C)) * 0.02).astype(np.float32),
        "b": (rng.standard_normal((N_SLOTS, C)) * 0.02).astype(np.float32),
    }
    loss, preds = kernel(**inputs)
    print("loss:", loss, "preds:", preds[:10])


# revision 40
# speedup vs baseline: 2.9960x; 1.0079x over previous
"""Trainium2 Bass kernel for slot-routed classifier head (moe_routing).

Reference computation (per token t, slot s = t % 30):
    logits = x[t] @ W[s] + b[s]            # [200]
    nll[t] = logsumexp(logits) - logits[id[t]]
    loss   = mean(nll over valid tokens);  preds[t] = argmax(logits)

Strategy:
  - Host: reorder X to slot-major transposed layout [30, 768, B_CORE] per core
    (data-parallel over the 4096 dialogs, 512 dialogs per core). This makes
    every matmul operand naturally laid out (contraction dim on partitions),
    so the device does zero transposes.
  - Device (per core): for each (slot, dialog-tile of 128):
      PE:  6 accumulating fp32 matmuls -> PSUM logits [128 tok, 200 cls]
      DVE: bias-add + PSUM evac (scalar_tensor_tensor)
           reduce_max -> m
           label gather: (iota == id) * logits, fused free-dim accum -> logit[id]
           argmax: (logits >= m) * (200 - iota), reduce_max -> 200 - argmax
      ACT: Exp with fused free-dim accum -> Z (no max-sub needed: |logits| < ~6)
    Tails: Ln(Z), nll = lnZ - label_logit, preds = 200 - mx; DMA out.
  - Host: mask invalid (-1) ids, fp64 sum for the scalar loss, gather preds.
"""

import os
from contextlib import ExitStack

import numpy as np

import concourse.bass as bass
import concourse.bacc as bacc
import concourse.mybir as mybir
import concourse.tile as tile
from concourse.bass_utils import run_bass_kernel_spmd

# Problem constants (hardcoded per contract)
N_SLOTS = 30
HIDDEN = 768
C = 200  # num labels
N_TOKENS = 122880
N_CORES = 8
B = N_TOKENS // N_SLOTS  # 4096 dialogs
P = 128
KC = HIDDEN // P  # 6 contraction chunks

F32 = mybir.dt.float32
F16 = mybir.dt.float16
I32 = mybir.dt.int32
AX = mybir.AxisListType
OP = mybir.AluOpType
AF = mybir.ActivationFunctionType

# fp16 hi/lo split matmul: logits are computed at scale WSCALE (W pre-scaled
# on host to dodge fp16 subnormals); exp/label paths rescale by 1/WSCALE.
WSCALE = 64.0


def _register_dve_ops():
    """Register three fused custom DVE ops (runtime registration — the table
    is generated per-NEFF at compile time, no firmware change needed):
      ANT_ADD_MAXRED: out = in0 + in1; accum = max(out)   (bias+evac+rowmax)
      ANT_LABEL_SUM:  out = (Idx == s0) ? in0 : 0; accum = sum  (label gather)
      ANT_ARGMAX_REV: out = (in0 >= s0) ? imm2 - Idx : 0; accum = max
                      (=> imm2 - first argmax index)
    """
    from operator import add as _add

    import concourse.dve_ops as dve_ops
    from concourse.dve_spec import (
        Spec, Src0, C0, C2, Zero, Idx, select, eq, maxx, lower,
    )
    from concourse.dve_spec import Src1
    from concourse.dve_ops import has_src1
    from concourse.dve_uop import DveOpSpec

    def _ref_add_maxred(in0, in1, c0, c1, c2):
        out = in0.astype(np.float32) + in1.astype(np.float32)
        acc = out.reshape(out.shape[0], -1).max(axis=-1, keepdims=True)
        return out, acc

    def _ref_label_sum(in0, in1, c0, c1, c2):
        f = in0.astype(np.float32).reshape(in0.shape[0], -1)
        idx = np.arange(f.shape[1], dtype=np.float32)[None, :]
        c0a = np.asarray(c0, dtype=np.float32).reshape(-1, 1)
        out = np.where(idx == c0a, f, np.float32(0.0))
        return out.reshape(in0.shape), out.sum(axis=-1, keepdims=True)

    def _ref_argmax_rev(in0, in1, c0, c1, c2):
        f = in0.astype(np.float32).reshape(in0.shape[0], -1)
        idx = np.arange(f.shape[1], dtype=np.float32)[None, :]
        c0a = np.asarray(c0, dtype=np.float32).reshape(-1, 1)
        out = np.where(f >= c0a, np.float32(c2) - idx, np.float32(0.0))
        return out.reshape(in0.shape), out.max(axis=-1, keepdims=True)

    specs = [
        ("ANT_ADD_MAXRED",
         Spec(body=Src0 + Src1, accum=maxx, reference=_ref_add_maxred)),
        ("ANT_LABEL_SUM",
         Spec(body=select(eq(Idx, C0), Src0, Zero), accum=_add,
              reference=_ref_label_sum)),
        ("ANT_ARGMAX_REV",
         Spec(body=select(Src0 >= C0, C2 - Idx, Zero), accum=maxx,
              reference=_ref_argmax_rev)),
    ]
    out = {}
    for name, spec in specs:
        if name in dve_ops._SUB_OPCODE_FOR_NAME:
            out[name] = next(o for o in dve_ops.OPS if o.name == name)
            continue
        opcode = dve_ops._CUSTOM_DVE_ROW_BASE + len(dve_ops.OPS)
        shas = {}
        for ver in ("v3", "v4"):
            try:
                u = lower(spec, ver=ver)
                shas[ver] = DveOpSpec(
                    name=name, opcode=opcode, uops=u, rd1_en=has_src1(spec)
                ).sha(ver)
            except Exception:
                pass
        op = dve_ops.DveOp(name, spec, subdim=False, uops_sha=shas)
        dve_ops.OPS.append(op)
        dve_ops._SUB_OPCODE_FOR_NAME[name] = opcode
        dve_ops.CUSTOM_DVE_SPECS[name] = spec
        out[name] = op
    return out


_DVE_OPS = _register_dve_ops()


def _build(b_core: int, s_core: int = N_SLOTS) -> bass.Bass:
    """Build the single-core SPMD program for a shard of b_core dialogs
    x s_core slots."""
    dt_n = b_core // P  # dialog tiles per core
    ncols = dt_n * s_core

    nc = bacc.Bacc("TRN2", target_bir_lowering=False, debug=False)

    # xw packs [Xhi | Xlo | W'hi | W'lo] (fp16) along the free dim:
    # one DMA (and one wait) per slot
    fw = 2 * b_core + 2 * C
    xw = nc.dram_tensor("xw", [s_core, HIDDEN, fw], F16, kind="ExternalInput").ap()
    bb = nc.dram_tensor("bb", [P, s_core, C], F32, kind="ExternalInput").ap()
    ids = nc.dram_tensor("ids", [P, ncols], F32, kind="ExternalInput").ap()
    nll = nc.dram_tensor("nll", [P, ncols], F32, kind="ExternalOutput").ap()
    preds = nc.dram_tensor("preds", [P, ncols], I32, kind="ExternalOutput").ap()

    with tile.TileContext(nc) as tc, ExitStack() as ctx:
        const_pool = ctx.enter_context(tc.tile_pool(name="const", bufs=1))
        xt_pool = ctx.enter_context(tc.tile_pool(name="xtp", bufs=3))
        psum_pool = ctx.enter_context(tc.tile_pool(name="ps", bufs=8, space="PSUM"))
        lg_pool = ctx.enter_context(tc.tile_pool(name="lg", bufs=8))
        e_pool = ctx.enter_context(tc.tile_pool(name="ep", bufs=4))
        j_pool = ctx.enter_context(tc.tile_pool(name="jp", bufs=8))

        # Constants / accumulators (persistent)
        bb_sb = const_pool.tile([P, s_core, C], F32)
        nc.sync.dma_start(bb_sb[:], bb[:])
        ids_sb = const_pool.tile([P, ncols], F32)
        nc.sync.dma_start(ids_sb[:], ids[:])

        # One-time wait absorbers: compute-op ISA structs encode only one sem
        # wait, so pre-touch each const DMA on DVE; later DVE ops then only
        # ever wait on the PE semaphore.
        warm = const_pool.tile([P, 4], F32)
        nc.vector.tensor_copy(warm[:, 0:1], bb_sb[:, 0, 0:1])
        nc.vector.tensor_copy(warm[:, 1:2], ids_sb[:, 0:1])

        m_all = const_pool.tile([P, ncols], F32)
        z_all = const_pool.tile([P, ncols], F32)
        lbl_all = const_pool.tile([P, ncols], F32)
        mx_all = const_pool.tile([P, ncols], F32)

        o_xlo = b_core
        o_whi = 2 * b_core
        o_wlo = 2 * b_core + C
        for s in range(s_core):
            xw_sb = xt_pool.tile([P, KC, fw], F16)
            nc.sync.dma_start(xw_sb[:], xw[s].rearrange("(k p) f -> p k f", p=P))

            for dti in range(dt_n):
                col = dti * s_core + s
                ps = psum_pool.tile([P, C], F32)
                for k in range(KC):
                    xhi = xw_sb[:, k, bass.ts(dti, P)]
                    xlo = xw_sb[:, k, o_xlo + dti * P : o_xlo + (dti + 1) * P]
                    whi = xw_sb[:, k, o_whi : o_whi + C]
                    wlo = xw_sb[:, k, o_wlo : o_wlo + C]
                    # logits*WSCALE = Xhi@W'hi + Xhi@W'lo + Xlo@W'hi
                    nc.tensor.matmul(ps[:], xhi, whi, start=(k == 0), stop=False)
                    nc.tensor.matmul(ps[:], xhi, wlo, start=False, stop=False)
                    nc.tensor.matmul(
                        ps[:], xlo, whi, start=False, stop=(k == KC - 1)
                    )
                # logits = psum + b_s; m = rowmax  (bias+evac+max in ONE op)
                lg = lg_pool.tile([P, C], F32)
                nc.vector._custom_dve(
                    _DVE_OPS["ANT_ADD_MAXRED"], out=lg[:], in0=ps[:],
                    in1=bb_sb[:, s, :], accum_out=m_all[:, col : col + 1],
                )
                # e = exp(logits); Z = sum(e) fused
                ej = e_pool.tile([P, C], F32)
                nc.scalar.activation(
                    ej[:], lg[:], AF.Exp, scale=1.0 / WSCALE,
                    accum_out=z_all[:, col : col + 1],
                )
                # label logit: sum((Idx == id) * logits) in ONE op
                j1 = j_pool.tile([P, C], F32)
                nc.vector._custom_dve(
                    _DVE_OPS["ANT_LABEL_SUM"], out=j1[:], in0=lg[:],
                    s0=ids_sb[:, col : col + 1],
                    accum_out=lbl_all[:, col : col + 1],
                )
                # argmax: max((logits >= m) ? 200 - Idx : 0) = 200 - argmax
                j2 = j_pool.tile([P, C], F32)
                nc.vector._custom_dve(
                    _DVE_OPS["ANT_ARGMAX_REV"], out=j2[:], in0=lg[:],
                    s0=m_all[:, col : col + 1], imm2=float(C),
                    accum_out=mx_all[:, col : col + 1],
                )

        # Tails  (lbl_all is at scale WSCALE; nll = lnZ - lbl/WSCALE)
        lnz = const_pool.tile([P, ncols], F32)
        nc.scalar.activation(lnz[:], z_all[:], AF.Ln)
        nll_sb = const_pool.tile([P, ncols], F32)
        nc.vector.scalar_tensor_tensor(
            out=nll_sb[:], in0=lbl_all[:], scalar=-1.0 / WSCALE, op0=OP.mult,
            in1=lnz[:], op1=OP.add,
        )
        preds_sb = const_pool.tile([P, ncols], I32)
        nc.vector.tensor_scalar(
            preds_sb[:], mx_all[:], -1.0, 200.0, op0=OP.mult, op1=OP.add
        )
        nc.sync.dma_start(nll[:], nll_sb[:])
        nc.sync.dma_start(preds[:], preds_sb[:])

    _legalize_waits(nc)  # drop provably-redundant waits (fewer event-sems)
    nc.finalize()  # Bacc pipeline: wait splitting via event sems + ISA codegen
    return nc


# ISA structs for compute ops encode a single sem-wait command; Tile's
# scheduler freely attaches several. Legalize: (1) drop waits on an engine's
# own sem that program order already satisfies, (2) hoist extra waits onto an
# earlier same-engine instruction with a free wait slot (safe: engines are
# in-order, so waiting earlier only strengthens the schedule).
_ONE_WAIT_OPS = {
    "Matmult", "Activation", "TensorScalarPtr", "TensorReduce", "TensorTensor",
    "TensorCopy", "TensorScalar", "Memset", "Iota", "TensorMaskReduce",
    "DMACopy", "CustomDveAnt", "ISA",
}


def _legalize_waits(nc: bass.Bass, one_wait_ops=_ONE_WAIT_OPS):
    import bass_rust
    for f in nc.m.functions:
        for bl in f.blocks:
            insns = list(bl.instructions)
            sem_updaters: dict[int, set] = {}
            sem_async: set[int] = set()  # sems inc'd at async DMA completion
            for ins in insns:
                si = ins.sync_info
                if si:
                    for u in si.on_update:
                        sem_updaters.setdefault(u.id, set()).add(ins.engine)
                        if ins.opcode == "DMACopy":
                            sem_async.add(u.id)

            # sems incremented exclusively by one engine's (sync) instructions
            sync_engine_sem: dict[int, object] = {}
            for sid, ups in sem_updaters.items():
                if len(ups) == 1 and sid not in sem_async:
                    sync_engine_sem[sid] = next(iter(ups))

            # Pass A: implication tables. For each sync-engine sem S, for each
            # inc index n, the max (other_sem -> wait_value) seen on S's
            # engine stream before the n-th inc completes.
            # impl[S] = list of (inc_index, {sem: maxval}) snapshots.
            running: dict[object, dict[int, int]] = {}
            impl: dict[int, list] = {}
            inc_count: dict[int, int] = {}
            for ins in insns:
                eng = ins.engine
                si = ins.sync_info
                if not si:
                    continue
                r = running.setdefault(eng, {})
                for w in si.on_wait:
                    if w.wait_mode == "sem-ge-imm":
                        if r.get(w.id, -1) < w.wait_value:
                            r[w.id] = w.wait_value
                for u in si.on_update:
                    if u.update_mode == "sem-inc" and sync_engine_sem.get(u.id) == eng:
                        inc_count[u.id] = inc_count.get(u.id, 0) + u.update_value
                        impl.setdefault(u.id, []).append(
                            (inc_count[u.id], dict(r))
                        )

            def implied(kept_waits, w):
                """Is wait w implied by any wait already kept?"""
                for k in kept_waits:
                    tab = impl.get(k.id)
                    if not tab or k.wait_mode != "sem-ge-imm":
                        continue
                    # snapshot at the largest inc index <= k.wait_value
                    snap = None
                    for n, d in tab:
                        if n <= k.wait_value:
                            snap = d
                        else:
                            break
                    if snap is not None and snap.get(w.id, -1) >= w.wait_value:
                        return True
                return False

            # Pass B: rewrite
            inc_seen: dict[int, int] = {}
            targets: dict = {}
            unplaced = 0
            for ins in insns:
                eng = ins.engine
                si = ins.sync_info
                waits = list(si.on_wait) if si else []
                updates = list(si.on_update) if si else []
                if ins.opcode in one_wait_ops and len(waits) > 1:
                    kept = []
                    for w in waits:
                        if (
                            sync_engine_sem.get(w.id) == eng
                            and w.wait_mode == "sem-ge-imm"
                            and inc_seen.get(w.id, 0) >= w.wait_value
                        ):
                            continue  # program order satisfies it
                        kept.append(w)
                    if len(kept) > 1:
                        primary = kept[:1]
                        for w in kept[1:]:
                            if not implied(primary, w):
                                primary.append(w)
                        kept = primary
                    # leftovers with >1 wait are split into preceding Drains
                    # at the JSON level (_split_residual_waits_in_json)
                    ins.sync_info = bass_rust.SyncInfo(on_wait=kept, on_update=updates)
                for u in updates:
                    if u.update_mode == "sem-inc":
                        inc_seen[u.id] = inc_seen.get(u.id, 0) + u.update_value
                if ins.opcode not in ("UnconditionalBranch", "Call"):
                    cur = ins.sync_info
                    if ins.opcode == "Drain" or not cur or len(cur.on_wait) == 0:
                        targets.setdefault(eng, []).append(ins)
                        if len(targets[eng]) > 64:
                            targets[eng] = targets[eng][-64:]
            if unplaced:
                import logging
                logging.warning(f"legalize_waits: {unplaced} waits could not be placed")


def _split_residual_waits_in_json(nc: bass.Bass):
    """Final walrus-facing fix: split any instruction still carrying >1 sem
    wait into preceding single-wait Drains on the same engine (pure wait
    sequencing — no reordering), and pin the patched JSON onto the instance
    so every downstream serialization (bass2jax, compile) uses it."""
    import orjson

    bir = orjson.loads(type(nc).to_json_bytes(nc))
    n = 0
    for f in bir["functions"]:
        for bl in f["blocks"]:
            out = []
            for ins in bl.get("instructions", []):
                si = ins.get("sync_info") or {}
                waits = si.get("on_wait") or []
                # raw-ISA instructions (custom DVE) cannot carry any inline
                # sem wait ("ISA wrong length"); others carry exactly one.
                keep = 0 if ins.get("opcode") == "ISA" else 1
                if len(waits) > keep:
                    for w in waits[: len(waits) - keep]:
                        n += 1
                        out.append({
                            "debug": ins.get("debug", 0),
                            "engine": ins["engine"],
                            "ins": [],
                            "outs": [],
                            "name": f"{ins['name']}-lgw{n}",
                            "opcode": "Drain",
                            "sync_info": {"on_update": [], "on_wait": [w]},
                        })
                    si = dict(si)
                    si["on_wait"] = waits[len(waits) - keep :]
                    ins = dict(ins)
                    ins["sync_info"] = si
                out.append(ins)
            bl["instructions"] = out
    data = orjson.dumps(bir)
    nc.to_json_bytes = lambda: data  # type: ignore[method-assign]


_NC_CACHE: dict = {}


def _get_nc(b_core: int, s_core: int = N_SLOTS) -> bass.Bass:
    key = (b_core, s_core)
    if key not in _NC_CACHE:
        _NC_CACHE[key] = _build(b_core, s_core)
    return _NC_CACHE[key]


def _prep_inputs(state_output, value_match_ids, W, b, n_cores: int):
    """Host-side sharding: core c handles dialog group c//2 and slot half c%2
    (halving per-core W traffic vs pure dialog-parallel). xw packs
    [Xhi | Xlo | W'hi | W'lo] fp16 per slot."""
    sp = 2 if n_cores % 2 == 0 and n_cores > 1 else 1  # slot split factor
    dgroups = n_cores // sp
    b_core = B // dgroups
    s_core = N_SLOTS // sp
    dt_n = b_core // P
    x3 = np.ascontiguousarray(state_output, dtype=np.float32).reshape(B, N_SLOTS, HIDDEN)
    # [30, 768, 4096]: xt[s, d, dialog]
    xt_full = x3.transpose(1, 2, 0)
    xhi_full = xt_full.astype(np.float16)
    xlo_full = (xt_full - xhi_full.astype(np.float32)).astype(np.float16)

    ws = np.asarray(W, dtype=np.float32) * np.float32(WSCALE)
    whi = ws.astype(np.float16)
    wlo = (ws - whi.astype(np.float32)).astype(np.float16)
    bbf = np.asarray(b, dtype=np.float32) * np.float32(WSCALE)

    ids2d = np.asarray(value_match_ids).reshape(B, N_SLOTS)
    in_maps = []
    for c in range(n_cores):
        g, h = c // sp, c % sp
        dsl = slice(g * b_core, (g + 1) * b_core)
        ssl = slice(h * s_core, (h + 1) * s_core)
        xw_c = np.empty((s_core, HIDDEN, 2 * b_core + 2 * C), dtype=np.float16)
        xw_c[:, :, :b_core] = xhi_full[ssl, :, dsl]
        xw_c[:, :, b_core : 2 * b_core] = xlo_full[ssl, :, dsl]
        xw_c[:, :, 2 * b_core : 2 * b_core + C] = whi[ssl]
        xw_c[:, :, 2 * b_core + C :] = wlo[ssl]
        bb_c = np.ascontiguousarray(
            np.broadcast_to(bbf[ssl], (P, s_core, C))
        )
        ids_c = ids2d[dsl, ssl].reshape(dt_n, P, s_core)
        ids_c = np.ascontiguousarray(
            ids_c.transpose(1, 0, 2).reshape(P, dt_n * s_core).astype(np.float32)
        )
        in_maps.append({"xw": xw_c, "bb": bb_c, "ids": ids_c})
    return in_maps


def _postprocess(results, value_match_ids, n_cores: int):
    sp = 2 if n_cores % 2 == 0 and n_cores > 1 else 1
    dgroups = n_cores // sp
    b_core = B // dgroups
    s_core = N_SLOTS // sp
    dt_n = b_core // P
    nll_tok = np.empty((B, N_SLOTS), dtype=np.float32)
    preds_tok = np.empty((B, N_SLOTS), dtype=np.int64)
    for c, r in enumerate(results):
        g, h = c // sp, c % sp
        dsl = slice(g * b_core, (g + 1) * b_core)
        ssl = slice(h * s_core, (h + 1) * s_core)
        nll_c = (
            np.asarray(r["nll"]).reshape(P, dt_n, s_core)
            .transpose(1, 0, 2).reshape(b_core, s_core)
        )
        preds_c = (
            np.asarray(r["preds"]).reshape(P, dt_n, s_core)
            .transpose(1, 0, 2).reshape(b_core, s_core)
        )
        nll_tok[dsl, ssl] = nll_c
        preds_tok[dsl, ssl] = preds_c

    ids2d = np.asarray(value_match_ids).reshape(B, N_SLOTS)
    valid = ids2d != -1
    count = int(valid.sum())
    if count > 0:
        loss = np.float32(
            np.where(valid, nll_tok, 0.0).sum(dtype=np.float64) / count
        )
    else:
        loss = np.float32(0.0)
    preds = np.where(valid, preds_tok, -1).reshape(-1).astype(np.int32)
    return loss, preds


def _run(inputs: dict, trace: bool = False):
    state_output = inputs["state_output"]
    value_match_ids = inputs["value_match_ids"]
    W = inputs["W"]
    b = inputs["b"]

    n_cores = N_CORES
    sp = 2 if n_cores % 2 == 0 and n_cores > 1 else 1
    nc = _get_nc(B // (n_cores // sp), N_SLOTS // sp)
    in_maps = _prep_inputs(state_output, value_match_ids, W, b, n_cores)
    res = run_bass_kernel_spmd(nc, in_maps, list(range(n_cores)), trace=trace)
    loss, preds = _postprocess(res.results, value_match_ids, n_cores)
    return (loss, preds), res


def kernel(**inputs) -> tuple:
    (loss, preds), _ = _run(inputs, trace=False)
    return loss, preds


if __name__ == "__main__":
    # Smoke test with random data
    rng = np.random.default_rng(0)
    inputs = {
        "state_output": rng.standard_normal((N_TOKENS, HIDDEN), dtype=np.float32),
        "op_ids": rng.integers(0, 3, size=(N_SLOTS,)),
        "value_match_ids": rng.integers(0, C, size=(N_TOKENS,)),
        "W": (rng.standard_normal((N_SLOTS, HIDDEN, C)) * 0.02).astype(np.float32),
        "b": (rng.standard_normal((N_SLOTS, C)) * 0.02).astype(np.float32),
    }
    loss, preds = kernel(**inputs)
    print("loss:", loss, "preds:", preds[:10])


# revision 41
# speedup vs baseline: 3.0749x; 1.0263x over previous
"""Trainium2 Bass kernel for slot-routed classifier head (moe_routing).

Reference computation (per token t, slot s = t % 30):
    logits = x[t] @ W[s] + b[s]            # [200]
    nll[t] = logsumexp(logits) - logits[id[t]]
    loss   = mean(nll over valid tokens);  preds[t] = argmax(logits)

Strategy:
  - Host: reorder X to slot-major transposed layout [30, 768, B_CORE] per core
    (data-parallel over the 4096 dialogs, 512 dialogs per core). This makes
    every matmul operand naturally laid out (contraction dim on partitions),
    so the device does zero transposes.
  - Device (per core): for each (slot, dialog-tile of 128):
      PE:  6 accumulating fp32 matmuls -> PSUM logits [128 tok, 200 cls]
      DVE: bias-add + PSUM evac (scalar_tensor_tensor)
           reduce_max -> m
           label gather: (iota == id) * logits, fused free-dim accum -> logit[id]
           argmax: (logits >= m) * (200 - iota), reduce_max -> 200 - argmax
      ACT: Exp with fused free-dim accum -> Z (no max-sub needed: |logits| < ~6)
    Tails: Ln(Z), nll = lnZ - label_logit, preds = 200 - mx; DMA out.
  - Host: mask invalid (-1) ids, fp64 sum for the scalar loss, gather preds.
"""

import os
from contextlib import ExitStack

import numpy as np

import concourse.bass as bass
import concourse.bacc as bacc
import concourse.mybir as mybir
import concourse.tile as tile
from concourse.bass_utils import run_bass_kernel_spmd

# Problem constants (hardcoded per contract)
N_SLOTS = 30
HIDDEN = 768
C = 200  # num labels
N_TOKENS = 122880
N_CORES = 8
B = N_TOKENS // N_SLOTS  # 4096 dialogs
P = 128
KC = HIDDEN // P  # 6 contraction chunks

F32 = mybir.dt.float32
F16 = mybir.dt.float16
I32 = mybir.dt.int32
AX = mybir.AxisListType
OP = mybir.AluOpType
AF = mybir.ActivationFunctionType

# fp16 hi/lo split matmul: logits are computed at scale WSCALE (W pre-scaled
# on host to dodge fp16 subnormals); exp/label paths rescale by 1/WSCALE.
WSCALE = 64.0


def _register_dve_ops():
    """Register three fused custom DVE ops (runtime registration — the table
    is generated per-NEFF at compile time, no firmware change needed):
      ANT_ADD_MAXRED: out = in0 + in1; accum = max(out)   (bias+evac+rowmax)
      ANT_LABEL_SUM:  out = (Idx == s0) ? in0 : 0; accum = sum  (label gather)
      ANT_ARGMAX_REV: out = (in0 >= s0) ? imm2 - Idx : 0; accum = max
                      (=> imm2 - first argmax index)
    """
    from operator import add as _add

    import concourse.dve_ops as dve_ops
    from concourse.dve_spec import (
        Spec, Src0, C0, C2, Zero, Idx, select, eq, maxx, lower,
    )
    from concourse.dve_spec import Src1
    from concourse.dve_ops import has_src1
    from concourse.dve_uop import DveOpSpec

    def _ref_add_maxred(in0, in1, c0, c1, c2):
        out = in0.astype(np.float32) + in1.astype(np.float32)
        acc = out.reshape(out.shape[0], -1).max(axis=-1, keepdims=True)
        return out, acc

    def _ref_label_sum(in0, in1, c0, c1, c2):
        f = in0.astype(np.float32).reshape(in0.shape[0], -1)
        idx = np.arange(f.shape[1], dtype=np.float32)[None, :]
        c0a = np.asarray(c0, dtype=np.float32).reshape(-1, 1)
        out = np.where(idx == c0a, f, np.float32(0.0))
        return out.reshape(in0.shape), out.sum(axis=-1, keepdims=True)

    def _ref_argmax_rev(in0, in1, c0, c1, c2):
        f = in0.astype(np.float32).reshape(in0.shape[0], -1)
        idx = np.arange(f.shape[1], dtype=np.float32)[None, :]
        c0a = np.asarray(c0, dtype=np.float32).reshape(-1, 1)
        out = np.where(f >= c0a, np.float32(c2) - idx, np.float32(0.0))
        return out.reshape(in0.shape), out.max(axis=-1, keepdims=True)

    specs = [
        ("ANT_ADD_MAXRED",
         Spec(body=Src0 + Src1, accum=maxx, reference=_ref_add_maxred)),
        ("ANT_LABEL_SUM",
         Spec(body=select(eq(Idx, C0), Src0, Zero), accum=_add,
              reference=_ref_label_sum)),
        ("ANT_ARGMAX_REV",
         Spec(body=select(Src0 >= C0, C2 - Idx, Zero), accum=maxx,
              reference=_ref_argmax_rev)),
    ]
    out = {}
    for name, spec in specs:
        if name in dve_ops._SUB_OPCODE_FOR_NAME:
            out[name] = next(o for o in dve_ops.OPS if o.name == name)
            continue
        opcode = dve_ops._CUSTOM_DVE_ROW_BASE + len(dve_ops.OPS)
        shas = {}
        for ver in ("v3", "v4"):
            try:
                u = lower(spec, ver=ver)
                shas[ver] = DveOpSpec(
                    name=name, opcode=opcode, uops=u, rd1_en=has_src1(spec)
                ).sha(ver)
            except Exception:
                pass
        op = dve_ops.DveOp(name, spec, subdim=False, uops_sha=shas)
        dve_ops.OPS.append(op)
        dve_ops._SUB_OPCODE_FOR_NAME[name] = opcode
        dve_ops.CUSTOM_DVE_SPECS[name] = spec
        out[name] = op
    return out


_DVE_OPS = _register_dve_ops()


def _build(b_core: int, s_core: int = N_SLOTS) -> bass.Bass:
    """Build the single-core SPMD program for a shard of b_core dialogs
    x s_core slots."""
    dt_n = b_core // P  # dialog tiles per core
    ncols = dt_n * s_core

    nc = bacc.Bacc("TRN2", target_bir_lowering=False, debug=False)

    # xw packs [Xhi | Xlo | W'hi | W'lo] (fp16) along the free dim:
    # one DMA (and one wait) per slot
    fw = 2 * b_core + 2 * C
    xw = nc.dram_tensor("xw", [s_core, HIDDEN, fw], F16, kind="ExternalInput").ap()
    bb = nc.dram_tensor("bb", [P, s_core, C], F32, kind="ExternalInput").ap()
    ids = nc.dram_tensor("ids", [P, ncols], F32, kind="ExternalInput").ap()
    nll = nc.dram_tensor("nll", [P, ncols], F32, kind="ExternalOutput").ap()
    preds = nc.dram_tensor("preds", [P, ncols], I32, kind="ExternalOutput").ap()

    with tile.TileContext(nc) as tc, ExitStack() as ctx:
        const_pool = ctx.enter_context(tc.tile_pool(name="const", bufs=1))
        xt_pool = ctx.enter_context(tc.tile_pool(name="xtp", bufs=3))
        psum_pool = ctx.enter_context(tc.tile_pool(name="ps", bufs=8, space="PSUM"))
        lg_pool = ctx.enter_context(tc.tile_pool(name="lg", bufs=8))
        e_pool = ctx.enter_context(tc.tile_pool(name="ep", bufs=4))
        j_pool = ctx.enter_context(tc.tile_pool(name="jp", bufs=8))

        # Constants / accumulators (persistent)
        bb_sb = const_pool.tile([P, s_core, C], F32)
        nc.sync.dma_start(bb_sb[:], bb[:])
        ids_sb = const_pool.tile([P, ncols], F32)
        nc.sync.dma_start(ids_sb[:], ids[:])

        # One-time wait absorbers: compute-op ISA structs encode only one sem
        # wait, so pre-touch each const DMA on DVE; later DVE ops then only
        # ever wait on the PE semaphore.
        warm = const_pool.tile([P, 4], F32)
        nc.vector.tensor_copy(warm[:, 0:1], bb_sb[:, 0, 0:1])
        nc.vector.tensor_copy(warm[:, 1:2], ids_sb[:, 0:1])

        m_all = const_pool.tile([P, ncols], F32)
        z_all = const_pool.tile([P, ncols], F32)
        lbl_all = const_pool.tile([P, ncols], F32)
        mx_all = const_pool.tile([P, ncols], F32)

        o_xlo = b_core
        o_whi = 2 * b_core
        o_wlo = 2 * b_core + C
        kh = KC // 2
        for s in range(s_core):
            # two half-depth DMAs: finer pipelining + parallel queues
            xw_a = xt_pool.tile([P, kh, fw], F16, tag="xwa")
            nc.sync.dma_start(
                xw_a[:], xw[s, : kh * P].rearrange("(k p) f -> p k f", p=P)
            )
            xw_b = xt_pool.tile([P, kh, fw], F16, tag="xwb")
            nc.sync.dma_start(
                xw_b[:], xw[s, kh * P :].rearrange("(k p) f -> p k f", p=P)
            )

            for dti in range(dt_n):
                col = dti * s_core + s
                ps = psum_pool.tile([P, C], F32)
                for k in range(KC):
                    xw_sb = xw_a if k < kh else xw_b
                    kk = k if k < kh else k - kh
                    xhi = xw_sb[:, kk, bass.ts(dti, P)]
                    xlo = xw_sb[:, kk, o_xlo + dti * P : o_xlo + (dti + 1) * P]
                    whi = xw_sb[:, kk, o_whi : o_whi + C]
                    wlo = xw_sb[:, kk, o_wlo : o_wlo + C]
                    # logits*WSCALE = Xhi@W'hi + Xhi@W'lo + Xlo@W'hi
                    nc.tensor.matmul(ps[:], xhi, whi, start=(k == 0), stop=False)
                    nc.tensor.matmul(ps[:], xhi, wlo, start=False, stop=False)
                    nc.tensor.matmul(
                        ps[:], xlo, whi, start=False, stop=(k == KC - 1)
                    )
                # logits = psum + b_s; m = rowmax  (bias+evac+max in ONE op)
                lg = lg_pool.tile([P, C], F32)
                nc.vector._custom_dve(
                    _DVE_OPS["ANT_ADD_MAXRED"], out=lg[:], in0=ps[:],
                    in1=bb_sb[:, s, :], accum_out=m_all[:, col : col + 1],
                )
                # e = exp(logits); Z = sum(e) fused
                ej = e_pool.tile([P, C], F32)
                nc.scalar.activation(
                    ej[:], lg[:], AF.Exp, scale=1.0 / WSCALE,
                    accum_out=z_all[:, col : col + 1],
                )
                # label logit: sum((Idx == id) * logits) in ONE op
                j1 = j_pool.tile([P, C], F32)
                nc.vector._custom_dve(
                    _DVE_OPS["ANT_LABEL_SUM"], out=j1[:], in0=lg[:],
                    s0=ids_sb[:, col : col + 1],
                    accum_out=lbl_all[:, col : col + 1],
                )
                # argmax: max((logits >= m) ? 200 - Idx : 0) = 200 - argmax
                j2 = j_pool.tile([P, C], F32)
                nc.vector._custom_dve(
                    _DVE_OPS["ANT_ARGMAX_REV"], out=j2[:], in0=lg[:],
                    s0=m_all[:, col : col + 1], imm2=float(C),
                    accum_out=mx_all[:, col : col + 1],
                )

        # Tails  (lbl_all is at scale WSCALE; nll = lnZ - lbl/WSCALE)
        lnz = const_pool.tile([P, ncols], F32)
        nc.scalar.activation(lnz[:], z_all[:], AF.Ln)
        nll_sb = const_pool.tile([P, ncols], F32)
        nc.vector.scalar_tensor_tensor(
            out=nll_sb[:], in0=lbl_all[:], scalar=-1.0 / WSCALE, op0=OP.mult,
            in1=lnz[:], op1=OP.add,
        )
        preds_sb = const_pool.tile([P, ncols], I32)
        nc.vector.tensor_scalar(
            preds_sb[:], mx_all[:], -1.0, 200.0, op0=OP.mult, op1=OP.add
        )
        nc.sync.dma_start(nll[:], nll_sb[:])
        nc.sync.dma_start(preds[:], preds_sb[:])

    _legalize_waits(nc)  # drop provably-redundant waits (fewer event-sems)
    nc.finalize()  # Bacc pipeline: wait splitting via event sems + ISA codegen
    return nc


# ISA structs for compute ops encode a single sem-wait command; Tile's
# scheduler freely attaches several. Legalize: (1) drop waits on an engine's
# own sem that program order already satisfies, (2) hoist extra waits onto an
# earlier same-engine instruction with a free wait slot (safe: engines are
# in-order, so waiting earlier only strengthens the schedule).
_ONE_WAIT_OPS = {
    "Matmult", "Activation", "TensorScalarPtr", "TensorReduce", "TensorTensor",
    "TensorCopy", "TensorScalar", "Memset", "Iota", "TensorMaskReduce",
    "DMACopy", "CustomDveAnt", "ISA",
}


def _legalize_waits(nc: bass.Bass, one_wait_ops=_ONE_WAIT_OPS):
    import bass_rust
    for f in nc.m.functions:
        for bl in f.blocks:
            insns = list(bl.instructions)
            sem_updaters: dict[int, set] = {}
            sem_async: set[int] = set()  # sems inc'd at async DMA completion
            for ins in insns:
                si = ins.sync_info
                if si:
                    for u in si.on_update:
                        sem_updaters.setdefault(u.id, set()).add(ins.engine)
                        if ins.opcode == "DMACopy":
                            sem_async.add(u.id)

            # sems incremented exclusively by one engine's (sync) instructions
            sync_engine_sem: dict[int, object] = {}
            for sid, ups in sem_updaters.items():
                if len(ups) == 1 and sid not in sem_async:
                    sync_engine_sem[sid] = next(iter(ups))

            # Pass A: implication tables. For each sync-engine sem S, for each
            # inc index n, the max (other_sem -> wait_value) seen on S's
            # engine stream before the n-th inc completes.
            # impl[S] = list of (inc_index, {sem: maxval}) snapshots.
            running: dict[object, dict[int, int]] = {}
            impl: dict[int, list] = {}
            inc_count: dict[int, int] = {}
            for ins in insns:
                eng = ins.engine
                si = ins.sync_info
                if not si:
                    continue
                r = running.setdefault(eng, {})
                for w in si.on_wait:
                    if w.wait_mode == "sem-ge-imm":
                        if r.get(w.id, -1) < w.wait_value:
                            r[w.id] = w.wait_value
                for u in si.on_update:
                    if u.update_mode == "sem-inc" and sync_engine_sem.get(u.id) == eng:
                        inc_count[u.id] = inc_count.get(u.id, 0) + u.update_value
                        impl.setdefault(u.id, []).append(
                            (inc_count[u.id], dict(r))
                        )

            def implied(kept_waits, w):
                """Is wait w implied by any wait already kept?"""
                for k in kept_waits:
                    tab = impl.get(k.id)
                    if not tab or k.wait_mode != "sem-ge-imm":
                        continue
                    # snapshot at the largest inc index <= k.wait_value
                    snap = None
                    for n, d in tab:
                        if n <= k.wait_value:
                            snap = d
                        else:
                            break
                    if snap is not None and snap.get(w.id, -1) >= w.wait_value:
                        return True
                return False

            # Pass B: rewrite
            inc_seen: dict[int, int] = {}
            targets: dict = {}
            unplaced = 0
            for ins in insns:
                eng = ins.engine
                si = ins.sync_info
                waits = list(si.on_wait) if si else []
                updates = list(si.on_update) if si else []
                if ins.opcode in one_wait_ops and len(waits) > 1:
                    kept = []
                    for w in waits:
                        if (
                            sync_engine_sem.get(w.id) == eng
                            and w.wait_mode == "sem-ge-imm"
                            and inc_seen.get(w.id, 0) >= w.wait_value
                        ):
                            continue  # program order satisfies it
                        kept.append(w)
                    if len(kept) > 1:
                        primary = kept[:1]
                        for w in kept[1:]:
                            if not implied(primary, w):
                                primary.append(w)
                        kept = primary
                    # leftovers with >1 wait are split into preceding Drains
                    # at the JSON level (_split_residual_waits_in_json)
                    ins.sync_info = bass_rust.SyncInfo(on_wait=kept, on_update=updates)
                for u in updates:
                    if u.update_mode == "sem-inc":
                        inc_seen[u.id] = inc_seen.get(u.id, 0) + u.update_value
                if ins.opcode not in ("UnconditionalBranch", "Call"):
                    cur = ins.sync_info
                    if ins.opcode == "Drain" or not cur or len(cur.on_wait) == 0:
                        targets.setdefault(eng, []).append(ins)
                        if len(targets[eng]) > 64:
                            targets[eng] = targets[eng][-64:]
            if unplaced:
                import logging
                logging.warning(f"legalize_waits: {unplaced} waits could not be placed")


def _split_residual_waits_in_json(nc: bass.Bass):
    """Final walrus-facing fix: split any instruction still carrying >1 sem
    wait into preceding single-wait Drains on the same engine (pure wait
    sequencing — no reordering), and pin the patched JSON onto the instance
    so every downstream serialization (bass2jax, compile) uses it."""
    import orjson

    bir = orjson.loads(type(nc).to_json_bytes(nc))
    n = 0
    for f in bir["functions"]:
        for bl in f["blocks"]:
            out = []
            for ins in bl.get("instructions", []):
                si = ins.get("sync_info") or {}
                waits = si.get("on_wait") or []
                # raw-ISA instructions (custom DVE) cannot carry any inline
                # sem wait ("ISA wrong length"); others carry exactly one.
                keep = 0 if ins.get("opcode") == "ISA" else 1
                if len(waits) > keep:
                    for w in waits[: len(waits) - keep]:
                        n += 1
                        out.append({
                            "debug": ins.get("debug", 0),
                            "engine": ins["engine"],
                            "ins": [],
                            "outs": [],
                            "name": f"{ins['name']}-lgw{n}",
                            "opcode": "Drain",
                            "sync_info": {"on_update": [], "on_wait": [w]},
                        })
                    si = dict(si)
                    si["on_wait"] = waits[len(waits) - keep :]
                    ins = dict(ins)
                    ins["sync_info"] = si
                out.append(ins)
            bl["instructions"] = out
    data = orjson.dumps(bir)
    nc.to_json_bytes = lambda: data  # type: ignore[method-assign]


_NC_CACHE: dict = {}


def _get_nc(b_core: int, s_core: int = N_SLOTS) -> bass.Bass:
    key = (b_core, s_core)
    if key not in _NC_CACHE:
        _NC_CACHE[key] = _build(b_core, s_core)
    return _NC_CACHE[key]


def _prep_inputs(state_output, value_match_ids, W, b, n_cores: int):
    """Host-side sharding: core c handles dialog group c//2 and slot half c%2
    (halving per-core W traffic vs pure dialog-parallel). xw packs
    [Xhi | Xlo | W'hi | W'lo] fp16 per slot."""
    sp = 2 if n_cores % 2 == 0 and n_cores > 1 else 1  # slot split factor
    dgroups = n_cores // sp
    b_core = B // dgroups
    s_core = N_SLOTS // sp
    dt_n = b_core // P
    x3 = np.ascontiguousarray(state_output, dtype=np.float32).reshape(B, N_SLOTS, HIDDEN)
    # [30, 768, 4096]: xt[s, d, dialog]
    xt_full = x3.transpose(1, 2, 0)
    xhi_full = xt_full.astype(np.float16)
    xlo_full = (xt_full - xhi_full.astype(np.float32)).astype(np.float16)

    ws = np.asarray(W, dtype=np.float32) * np.float32(WSCALE)
    whi = ws.astype(np.float16)
    wlo = (ws - whi.astype(np.float32)).astype(np.float16)
    bbf = np.asarray(b, dtype=np.float32) * np.float32(WSCALE)

    ids2d = np.asarray(value_match_ids).reshape(B, N_SLOTS)
    in_maps = []
    for c in range(n_cores):
        g, h = c // sp, c % sp
        dsl = slice(g * b_core, (g + 1) * b_core)
        ssl = slice(h * s_core, (h + 1) * s_core)
        xw_c = np.empty((s_core, HIDDEN, 2 * b_core + 2 * C), dtype=np.float16)
        xw_c[:, :, :b_core] = xhi_full[ssl, :, dsl]
        xw_c[:, :, b_core : 2 * b_core] = xlo_full[ssl, :, dsl]
        xw_c[:, :, 2 * b_core : 2 * b_core + C] = whi[ssl]
        xw_c[:, :, 2 * b_core + C :] = wlo[ssl]
        bb_c = np.ascontiguousarray(
            np.broadcast_to(bbf[ssl], (P, s_core, C))
        )
        ids_c = ids2d[dsl, ssl].reshape(dt_n, P, s_core)
        ids_c = np.ascontiguousarray(
            ids_c.transpose(1, 0, 2).reshape(P, dt_n * s_core).astype(np.float32)
        )
        in_maps.append({"xw": xw_c, "bb": bb_c, "ids": ids_c})
    return in_maps


def _postprocess(results, value_match_ids, n_cores: int):
    sp = 2 if n_cores % 2 == 0 and n_cores > 1 else 1
    dgroups = n_cores // sp
    b_core = B // dgroups
    s_core = N_SLOTS // sp
    dt_n = b_core // P
    nll_tok = np.empty((B, N_SLOTS), dtype=np.float32)
    preds_tok = np.empty((B, N_SLOTS), dtype=np.int64)
    for c, r in enumerate(results):
        g, h = c // sp, c % sp
        dsl = slice(g * b_core, (g + 1) * b_core)
        ssl = slice(h * s_core, (h + 1) * s_core)
        nll_c = (
            np.asarray(r["nll"]).reshape(P, dt_n, s_core)
            .transpose(1, 0, 2).reshape(b_core, s_core)
        )
        preds_c = (
            np.asarray(r["preds"]).reshape(P, dt_n, s_core)
            .transpose(1, 0, 2).reshape(b_core, s_core)
        )
        nll_tok[dsl, ssl] = nll_c
        preds_tok[dsl, ssl] = preds_c

    ids2d = np.asarray(value_match_ids).reshape(B, N_SLOTS)
    valid = ids2d != -1
    count = int(valid.sum())
    if count > 0:
        loss = np.float32(
            np.where(valid, nll_tok, 0.0).sum(dtype=np.float64) / count
        )
    else:
        loss = np.float32(0.0)
    preds = np.where(valid, preds_tok, -1).reshape(-1).astype(np.int32)
    return loss, preds


def _run(inputs: dict, trace: bool = False):
    state_output = inputs["state_output"]
    value_match_ids = inputs["value_match_ids"]
    W = inputs["W"]
    b = inputs["b"]

    n_cores = N_CORES
    sp = 2 if n_cores % 2 == 0 and n_cores > 1 else 1
    nc = _get_nc(B // (n_cores // sp), N_SLOTS // sp)
    in_maps = _prep_inputs(state_output, value_match_ids, W, b, n_cores)
    res = run_bass_kernel_spmd(nc, in_maps, list(range(n_cores)), trace=trace)
    loss, preds = _postprocess(res.results, value_match_ids, n_cores)
    return (loss, preds), res


def kernel(**inputs) -> tuple:
    (loss, preds), _ = _run(inputs, trace=False)
    return loss, preds


if __name__ == "__main__":
    # Smoke test with random data
    rng = np.random.default_rng(0)
    inputs = {
        "state_output": rng.standard_normal((N_TOKENS, HIDDEN), dtype=np.float32),
        "op_ids": rng.integers(0, 3, size=(N_SLOTS,)),
        "value_match_ids": rng.integers(0, C, size=(N_TOKENS,)),
        "W": (rng.standard_normal((N_SLOTS, HIDDEN, C)) * 0.02).astype(np.float32),
        "b": (rng.standard_normal((N_SLOTS, C)) * 0.02).astype(np.float32),
    }
    loss, preds = kernel(**inputs)
    print("loss:", loss, "preds:", preds[:10])
